# revision 24
# baseline (speedup 1.0000x reference)
"""Trainium2 distributed kernel for nn_AttentionFusion (BEV temporal+spatial attention).

Full computation on device across 8 NeuronCores, zero cross-core communication.

Sharding: 2x4 grid of core blocks (50x25 grid pixels per core). Each core
processes its block PLUS a 2-pixel halo (54x29 local region, out-of-grid
pixels zero) so the spatial neighbor windows are always core-local
(redundant temporal compute on the halo instead of a collective).

Per core:
  phase 1 (13 chunks of 128 px): temporal attention. x arrives channel-major
    (host pre-transposed bf16). h_t is never materialized: tWo is composed
    into the spatial projections on the host; k/v biases cancel or fold
    (softmax shift invariance + sum(p)=1). Writes a local kv table
    [1664, 768] = [k padded to 64/head | v] to DRAM.
  phase 2 (25 chunks of 10x5 queries): spatial window attention. The 126-px
    (14x9) window k is fetched channel-major with one transpose-mode
    dma_gather, v pixel-major with a second gather; scores/ctx are dense
    per-head matmuls, masked by a host-precomputed band mask. exp without
    max-subtraction (scores are tiny); softmax denominator via a ones-column
    matmul fused into the ctx pass.
  phase 3 (10 chunks): output projection.

Self-contained: only needs the container toolchain at /opt/trn_rl_repo.
"""

import math
import os
import sys

import numpy as np

sys.path.insert(0, "/opt/trn_rl_repo")

import ml_dtypes  # noqa: E402

import concourse.bass as bass  # noqa: E402
import concourse.bacc as bacc  # noqa: E402
import concourse.mybir as mybir  # noqa: E402
import concourse.tile as tile  # noqa: E402

F32 = mybir.dt.float32
FP8 = mybir.dt.float8e4
BF16 = mybir.dt.bfloat16
I16 = mybir.dt.int16
AX = mybir.AxisListType
ALU = mybir.AluOpType
ACTF = mybir.ActivationFunctionType

# Problem constants
N_FULL = 10000
GRID = 100
T = 5
C = 256
NH = 8
DK = 32
CORES = 8
CR, CC_ = 2, 4             # core grid 2 x 4
BR, BC = 50, 25            # block rows/cols per core
NLOC = BR * BC             # 1250 real pixels per core
HR, HC = BR + 4, BC + 4    # 54 x 29 local region (with halo)
NH_PIX = HR * HC           # 1566
NP = 128
G = (NH_PIX + NP - 1) // NP        # 13 projection chunks
NPAD = G * NP                      # 1664
GO = 10                            # output-projection chunks
NPO = NLOC // GO                   # 125
# spatial chunks: 10 rows x 5 cols of queries
SQR, SQC = 10, 5
NS_R, NS_C = BR // SQR, BC // SQC  # 5 x 5 = 25
NS = NS_R * NS_C
NQ = SQR * SQC             # 50
WR, WC = SQR + 4, SQC + 4  # 14 x 9
NW = WR * WC               # 126
KROW = 384                 # k section: 3 heads per 128-col group at offsets {0,32,64}
# head slot hh (in kq32 tables) -> standard head: hh = 3*(h%3) + h//3
PERM = [0, 3, 6, 1, 4, 7, 2, 5]
VROW = 256
ROW = KROW + VROW          # 768

_CACHE = {}


def _bf16(a):
    return np.asarray(a, dtype=ml_dtypes.bfloat16)


def _pad_cols(w):
    """[C, 256] -> [C, 384]: head h (32 cols) at 128*(h//3) + 32*(h%3)."""
    w = w.reshape(-1, NH * DK)
    out = np.zeros((w.shape[0], KROW), np.float32)
    for h in range(NH):
        base = 128 * (h // 3) + 32 * (h % 3)
        out[:, base : base + DK] = w[:, DK * h : DK * (h + 1)]
    return out


def _build_graph():
    nc = bacc.Bacc(
        "TRN2",
        target_bir_lowering=False,
        debug=False,
        enable_asserts=False,
        num_devices=CORES,
    )

    # ---------------- I/O ----------------
    x_d = nc.dram_tensor("x", [G, 128, 2 * T * NP], FP8, kind="ExternalInput")
    w_d = {
        "wkv_t": nc.dram_tensor("wkv_t", [2, 128, 2 * C], BF16, kind="ExternalInput"),
        "wq_t": nc.dram_tensor("wq_t", [2, 128, C], BF16, kind="ExternalInput"),
        # spatial q projection, transposed output layout, head-padded [2,128,512]
        "wq_e": nc.dram_tensor("wq_e", [2, 128, KROW], BF16, kind="ExternalInput"),
        # fused (k_padded | v) projection [2, 128, 768]
        "wkv_e": nc.dram_tensor("wkv_e", [2, 128, ROW], BF16, kind="ExternalInput"),
        "wo_s": nc.dram_tensor("wo_s", [2, 128, C], BF16, kind="ExternalInput"),
    }
    b_d = {
        "bq_t": nc.dram_tensor("bq_t", [1, C], BF16, kind="ExternalInput"),
        "bq_e": nc.dram_tensor("bq_e", [1, KROW], BF16, kind="ExternalInput"),
        "bo_e": nc.dram_tensor("bo_e", [1, C], BF16, kind="ExternalInput"),
    }
    ident_d = nc.dram_tensor("ident", [128, 128], BF16, kind="ExternalInput")
    ones1_d = nc.dram_tensor("ones1", [1, 128], BF16, kind="ExternalInput")
    masks_d = nc.dram_tensor("masks", [NW, NS * NQ], BF16, kind="ExternalInput")
    out_d = nc.dram_tensor("out", [NLOC, C], BF16, kind="ExternalOutput")

    with tile.TileContext(nc) as tc:
        with (
            tc.tile_pool(name="const", bufs=1) as cpool,
            tc.tile_pool(name="dram", bufs=1, space="DRAM") as dpool,
            tc.tile_pool(name="sb", bufs=4) as sb,
            tc.tile_pool(name="pkv", bufs=2, space="PSUM") as pkv,
            tc.tile_pool(name="pqt", bufs=1, space="PSUM") as pqt,
            tc.tile_pool(name="pkv1", bufs=1, space="PSUM") as pkv1,
            tc.tile_pool(name="ppv", bufs=1, space="PSUM") as ppv,
            tc.tile_pool(name="ptp", bufs=1, space="PSUM") as ptp,
            tc.tile_pool(name="psc", bufs=1, space="PSUM") as psc,
            tc.tile_pool(name="pcx", bufs=1, space="PSUM") as pcx,
        ):
            v_dram = dpool.tile([NPAD, VROW], BF16, tag="v_dram")

            # ---------- constants ----------
            w_sb = {}
            for n, d in w_d.items():
                t_ = cpool.tile([128, 2, d.shape[2]], BF16, tag=f"w_{n}")
                nc.sync.dma_start(t_[:], d.ap().rearrange("a p c -> p a c"))
                w_sb[n] = t_
            b_sb = {}
            for n, d in b_d.items():
                t_ = cpool.tile([1, d.shape[1]], BF16, tag=f"b_{n}")
                nc.sync.dma_start(t_[:], d.ap())
                b_sb[n] = t_
            ident = cpool.tile([128, 128], BF16, tag="ident")
            nc.sync.dma_start(ident[:], ident_d.ap())
            ones1 = cpool.tile([1, 128], BF16, tag="ones1")
            nc.sync.dma_start(ones1[:], ones1_d.ap())
            onesw = cpool.tile([128, 1], BF16, tag="onesw")
            nc.vector.memset(onesw[:], 1.0)
            masks = cpool.tile([128, NS, NQ], BF16, tag="masks")
            nc.sync.dma_start(
                masks[0:NW, :, :], masks_d.ap().rearrange("w (s q) -> w s q", s=NS)
            )
            kq32 = cpool.tile([32, NH, 2, NH_PIX], BF16, tag="kq32")
            cT_all = cpool.tile([128, 2, NLOC], BF16, tag="cT_all")

            def bias_mm(psum_t, b_key, n_out, rows):
                nc.tensor.matmul(
                    psum_t,
                    ones1[0:1, 0:rows],
                    b_sb[b_key][0:1, 0:n_out],
                    start=False,
                    stop=True,
                )

            # ================= PHASE 1: temporal =================
            for g in range(G):
                xg = sb.tile([128, 2, T, NP], BF16, tag="xg")
                nc.gpsimd.dma_start(
                    xg[:], x_d.ap()[g].rearrange("p (a t n) -> p a t n", a=2, t=T)
                )

                kv_sb = sb.tile([NP, T, 2 * C], BF16, tag="kv_sb")
                for t in range(T):
                    kvp = pkv.tile([NP, 2 * C], F32, tag="kvp")
                    for cc in range(2):
                        nc.tensor.matmul(
                            kvp[:],
                            xg[:, cc, t, :],
                            w_sb["wkv_t"][:, cc, :],
                            start=(cc == 0),
                            stop=(cc == 1),
                        )
                    nc.scalar.copy(kv_sb[:, t, :], kvp[:])
                k_sb = kv_sb[:, :, 0:C]
                v_sb = kv_sb[:, :, C : 2 * C]

                qp = pkv.tile([NP, 2 * C], F32, tag="kvp", name="qp")[:, 0:C]
                for cc in range(2):
                    nc.tensor.matmul(
                        qp[:], xg[:, cc, T - 1, :], w_sb["wq_t"][:, cc, :],
                        start=(cc == 0), stop=False,
                    )
                bias_mm(qp[:], "bq_t", C, NP)
                q_sb = sb.tile([NP, C], BF16, tag="q_sb")
                nc.scalar.copy(q_sb[:], qp[:])

                # scores over t (no max subtraction: |s| < 1)
                prod = sb.tile([NP, T, NH, DK], BF16, tag="prod")
                nc.vector.tensor_mul(
                    prod[:],
                    k_sb.rearrange("p t (h d) -> p t h d", h=NH),
                    q_sb[:].rearrange("p (h d) -> p h d", h=NH)
                    .unsqueeze(1)
                    .broadcast_to((NP, T, NH, DK)),
                )
                s_t = sb.tile([NP, T, NH], BF16, tag="s_t")
                with nc.allow_low_precision(reason="temporal scores bf16"):
                    nc.vector.tensor_reduce(s_t[:], prod[:], axis=AX.X, op=ALU.add)
                es = sb.tile([NP, T, NH], F32, tag="es")
                nc.scalar.activation(es[:], s_t[:], ACTF.Exp)
                tsum = sb.tile([NP, NH], F32, tag="tsum")
                nc.vector.tensor_reduce(
                    tsum[:], es[:].rearrange("p t h -> p h t"), axis=AX.X, op=ALU.add
                )
                rinv = sb.tile([NP, NH], F32, tag="trinv")
                nc.vector.reciprocal(rinv[:], tsum[:])
                p_t = sb.tile([NP, T, NH], BF16, tag="p_t")
                nc.vector.tensor_mul(
                    p_t[:], es[:], rinv[:].unsqueeze(1).broadcast_to((NP, T, NH))
                )

                wv = sb.tile([NP, T, C], BF16, tag="wv")
                nc.vector.tensor_mul(
                    wv[:].rearrange("p t (h d) -> p t h d", h=NH),
                    v_sb.rearrange("p t (h d) -> p t h d", h=NH),
                    p_t[:].unsqueeze(3).broadcast_to((NP, T, NH, DK)),
                )
                c1 = sb.tile([NP, 2, C], BF16, tag="c1")
                nc.vector.tensor_add(c1[:], wv[:, 0:2, :], wv[:, 2:4, :])
                c2 = sb.tile([NP, C], BF16, tag="c2")
                nc.vector.tensor_add(c2[:], c1[:, 0, :], c1[:, 1, :])
                ctx = sb.tile([NP, C], BF16, tag="ctx")
                nc.vector.tensor_add(ctx[:], c2[:], wv[:, 4, :])

                # ctxT (channel-major)
                ctp = ptp.tile([128, 2, NP], BF16, tag="ctp")
                for cc in range(2):
                    nc.tensor.transpose(
                        ctp[:, cc, :], ctx[:, cc * 128 : (cc + 1) * 128],
                        ident[0:NP, 0:NP],
                    )
                ctxT = sb.tile([128, 2, NP], BF16, tag="ctxT")
                nc.scalar.copy(ctxT[:], ctp[:])

                # q'T via transposed projection: psum [128, 4, 128]
                qtp = pqt.tile([128, 3, NP], F32, tag="qtp")
                for grp in range(3):
                    for cc in range(2):
                        nc.tensor.matmul(
                            qtp[:, grp, :],
                            w_sb["wq_e"][:, cc, 128 * grp : 128 * (grp + 1)],
                            ctxT[:, cc, :],
                            start=(cc == 0),
                            stop=False,
                        )
                    nc.tensor.matmul(
                        qtp[:, grp, :],
                        b_sb["bq_e"][0:1, 128 * grp : 128 * (grp + 1)],
                        ones1[0:1, 0:NP],
                        start=False,
                        stop=True,
                    )
                kq_tmp = sb.tile([128, 3, 2, NP], BF16, tag="kq_tmp")
                nc.vector.tensor_copy(kq_tmp[:, :, 1, :], qtp[:])

                # k1 transposed projection
                ktp = pkv1.tile([128, 3, NP], F32, tag="kv1")
                for grp in range(3):
                    for cc in range(2):
                        nc.tensor.matmul(
                            ktp[:, grp, :],
                            w_sb["wkv_e"][:, cc, 128 * grp : 128 * (grp + 1)],
                            ctxT[:, cc, :],
                            start=(cc == 0),
                            stop=(cc == 1),
                        )
                nc.scalar.copy(kq_tmp[:, :, 0, :], ktp[:])
                # head regroup: partitions [32m:32m+32) -> head slots [3m:3m+3)
                glo = g * NP
                hi = min(NH_PIX - glo, NP)
                if hi > 0:
                    for m in range(3):
                        nc.sync.dma_start(
                            kq32[:, 3 * m : min(3 * m + 3, NH), :, glo : glo + hi],
                            kq_tmp[32 * m : 32 * m + 32, 0 : (3 if m < 2 else 2), :, 0:hi],
                        )
                vp = ppv.tile([NP, VROW], F32, tag="vp")
                for cc in range(2):
                    nc.tensor.matmul(
                        vp[:], ctxT[:, cc, :], w_sb["wkv_e"][:, cc, KROW:ROW],
                        start=(cc == 0), stop=(cc == 1),
                    )
                v1_sb = sb.tile([NP, VROW], BF16, tag="v1_sb")
                nc.scalar.copy(v1_sb[:], vp[:])
                nc.gpsimd.dma_start(
                    v_dram[g * NP : (g + 1) * NP, :], v1_sb[:]
                )

            PH = os.environ.get("KERNEL_PHASES", "123")
            # ================= PHASE 2: spatial =================
            for s in (range(NS) if "2" in PH else []):
                r0, c0 = (s // NS_C) * SQR, (s % NS_C) * SQC
                vw = sb.tile([NW, VROW], BF16, tag="vw")
                nc.gpsimd.dma_start(
                    vw[:],
                    v_dram[0:NH_PIX, :].rearrange("(r c) x -> r c x", r=HR)[
                        r0 : r0 + WR, c0 : c0 + WC, :
                    ],
                )
                kqv = kq32[:].rearrange("p h t (r c) -> p h t r c", r=HR)
                kTw = sb.tile([32, NH, NW], BF16, tag="kTw")
                nc.vector.tensor_copy(
                    kTw[:].rearrange("p h (r c) -> p h r c", r=WR),
                    kqv[:, :, 0, r0 : r0 + WR, c0 : c0 + WC],
                )
                qw = sb.tile([32, NH, NQ], BF16, tag="qw")
                nc.vector.tensor_copy(
                    qw[:].rearrange("p h (r c) -> p h r c", r=SQR),
                    kqv[:, :, 1, 2 + r0 : 2 + r0 + SQR, 2 + c0 : 2 + c0 + SQC],
                )

                LVL = int(os.environ.get("KERNEL_P2LVL", "9"))
                if LVL < 1:
                    continue
                sc = psc.tile([NW, NH, NQ], F32, tag="sc")
                for h in range(NH):
                    nc.tensor.matmul(
                        sc[:, h, :],
                        kTw[:, h, :],
                        qw[:, h, :],
                        start=True,
                        stop=True,
                    )
                if LVL < 2:
                    continue
                E = sb.tile([NW, NH, NQ], BF16, tag="E")
                nc.scalar.activation(E[:], sc[:], ACTF.Exp)
                E2 = sb.tile([NW, NH, NQ], BF16, tag="E2")
                nc.gpsimd.tensor_mul(
                    E2[:],
                    E[:],
                    masks[0:NW, s, :].unsqueeze(1).broadcast_to((NW, NH, NQ)),
                )

                if LVL < 3:
                    continue
                cx = pcx.tile([NQ, NH, DK + 1], F32, tag="cx")
                for h in range(NH):
                    nc.tensor.matmul(
                        cx[:, h, 0:DK],
                        E2[:, h, :],
                        vw[0:NW, DK * PERM[h] : DK * PERM[h] + DK],
                        start=True,
                        stop=True,
                    )
                    nc.tensor.matmul(
                        cx[:, h, DK : DK + 1],
                        E2[:, h, :],
                        onesw[0:NW, :],
                        start=True,
                        stop=True,
                    )
                srinv = sb.tile([NQ, NH], F32, tag="srinv")
                nc.vector.reciprocal(srinv[:], cx[:, :, DK])
                ctxn = sb.tile([NQ, C], BF16, tag="ctxn")
                nc.vector.tensor_mul(
                    ctxn[:].rearrange("q (h d) -> q h d", h=NH),
                    cx[:, :, 0:DK],
                    srinv[:].unsqueeze(2).broadcast_to((NQ, NH, DK)),
                )
                if LVL < 4:
                    continue
                ntp = ptp.tile([128, 2, NQ], BF16, tag="ctp")
                for cc in range(2):
                    nc.tensor.transpose(
                        ntp[:, cc, :], ctxn[:, cc * 128 : (cc + 1) * 128],
                        ident[0:NQ, 0:NQ],
                    )
                csel = cT_all[:, :, :].rearrange(
                    "p a (r c) -> p a r c", r=BR
                )[:, :, r0 : r0 + SQR, c0 : c0 + SQC]
                nc.vector.tensor_copy(
                    csel[:],
                    ntp[:].rearrange("p a (r c) -> p a r c", r=SQR),
                )

            # ================= PHASE 3: output proj =================
            if "3" not in PH:
                zz = sb.tile([NPO, C], BF16, tag="o_sb", name="zz")
                nc.vector.memset(zz[:], 0.0)
                nc.gpsimd.dma_start(out_d.ap()[0:NPO, :], zz[:])
            for g in (range(GO) if "3" in PH else []):
                op = pkv.tile([NPO, 2 * C], F32, tag="kvp", name="op")[:, 0:C]
                for cc in range(2):
                    nc.tensor.matmul(
                        op[:], cT_all[:, cc, g * NPO : (g + 1) * NPO],
                        w_sb["wo_s"][:, cc, :],
                        start=(cc == 0), stop=False,
                    )
                bias_mm(op[:], "bo_e", C, NPO)
                o_sb = sb.tile([NPO, C], BF16, tag="o_sb")
                nc.scalar.copy(o_sb[:], op[:])
                nc.gpsimd.dma_start(out_d.ap()[g * NPO : (g + 1) * NPO, :], o_sb[:])

    nc.compile()
    return nc


def _prep_weights(inputs):
    """Host-side weight transforms (all small)."""
    scale = 1.0 / math.sqrt(DK)
    f = lambda k: np.asarray(inputs[k], np.float32)
    tWq, tbq = f("t_Wq") * scale, f("t_bq") * scale
    tWk = f("t_Wk")
    tWv, tbv = f("t_Wv"), f("t_bv")
    tWo, tbo = f("t_Wo"), f("t_bo")
    sWq, sbq = f("s_Wq"), f("s_bq")
    sWk = f("s_Wk")
    sWv, sbv = f("s_Wv"), f("s_bv")
    sWo, sbo = f("s_Wo"), f("s_bo")

    hb = tbv @ tWo + tbo                    # constant part of h_t
    Wq_eff = (tWo @ sWq) * scale
    bq_eff = (hb @ sWq + sbq) * scale
    Wk_eff = tWo @ sWk                      # k bias dropped (softmax-invariant)
    Wv_eff = tWo @ sWv
    cv = hb @ sWv + sbv                     # constant part of v1
    bo_eff = sbo + cv @ sWo

    cm = {
        "wkv_t": _bf16(np.concatenate([tWk, tWv], axis=1).reshape(2, 128, 2 * C)),
        "wq_t": _bf16(tWq.reshape(2, 128, C)),
        "wq_e": _bf16(_pad_cols(Wq_eff).reshape(2, 128, KROW)),
        "wkv_e": _bf16(
            np.concatenate([_pad_cols(Wk_eff), Wv_eff], axis=1).reshape(2, 128, ROW)
        ),
        "wo_s": _bf16(
            sWo.reshape(NH, DK, C)[PERM].reshape(2, 128, C)
        ),
        "bq_t": _bf16(tbq.reshape(1, C)),
        "bq_e": _bf16(_pad_cols(bq_eff).reshape(1, KROW)),
        "bo_e": _bf16(bo_eff.reshape(1, C)),
        "ident": _bf16(np.eye(128, dtype=np.float32)),
        "ones1": _bf16(np.ones((1, 128), np.float32)),
    }
    return cm


def _prep_geometry():
    """Per-core gather indices (local coords) and window masks."""
    masks = np.zeros((CORES, NW, NS * NQ), np.float32)
    for R in range(CR):
        for C4 in range(CC_):
            core = R * CC_ + C4
            for s in range(NS):
                r0, c0 = (s // NS_C) * SQR, (s % NS_C) * SQC
                gr0, gc0 = BR * R + r0, BC * C4 + c0
                wr = gr0 - 2 + np.arange(WR)          # global window rows
                wc = gc0 - 2 + np.arange(WC)
                valid = (wr[:, None] >= 0) & (wr[:, None] < GRID) & \
                        (wc[None, :] >= 0) & (wc[None, :] < GRID)
                qr = gr0 + np.arange(SQR)
                qc = gc0 + np.arange(SQC)
                qrc = np.clip(qr, 2, GRID - 3)
                qcc = np.clip(qc, 2, GRID - 3)
                mrow = (np.abs(wr[:, None] - qrc[None, :]) <= 2)
                mcol = (np.abs(wc[:, None] - qcc[None, :]) <= 2)
                m = (mrow[:, None, :, None] & mcol[None, :, None, :] &
                     valid[:, :, None, None])
                masks[core, :, s * NQ : (s + 1) * NQ] = m.reshape(NW, NQ)
    return _bf16(masks)


def _prep_x(x):
    """x [10000, 5, 256] f32 -> per-core halo-extended channel-major bf16
    chunks [8, G, 128, 2*T*128]."""
    xb = np.asarray(x, np.float32).astype(ml_dtypes.float8_e4m3).reshape(
        GRID, GRID, T, C
    )
    xp = np.zeros((GRID + 4, GRID + 4, T, C), dtype=xb.dtype)
    xp[2 : 2 + GRID, 2 : 2 + GRID] = xb
    out = np.zeros((CORES, G, 128, 2 * T * NP), dtype=xb.dtype)
    for R in range(CR):
        for C4 in range(CC_):
            core = R * CC_ + C4
            blk = xp[BR * R : BR * R + HR, BC * C4 : BC * C4 + HC]  # [54,29,T,C]
            flat = blk.reshape(NH_PIX, T, C)
            flat = np.concatenate(
                [flat, np.zeros((NPAD - NH_PIX, T, C), dtype=xb.dtype)], axis=0
            )
            v = flat.reshape(G, NP, T, 2, 128)
            v = v.transpose(0, 4, 3, 2, 1)      # (g, ch, cc, t, px)
            out[core] = v.reshape(G, 128, 2 * T * NP)
    return out


def _unprep_out(res_list):
    """[8][1250, 256] bf16 -> [10000, 1, 256] f32 global row-major."""
    o = np.stack([np.asarray(r) for r in res_list], axis=0).astype(np.float32)
    v = o.reshape(CR, CC_, BR, BC, C)
    v = v.transpose(0, 2, 1, 3, 4)
    return np.ascontiguousarray(v.reshape(N_FULL, 1, C))


def _make_in_maps(inputs):
    cm = _prep_weights(inputs)
    if "geom" not in _CACHE:
        _CACHE["geom"] = _prep_geometry()
    masks = _CACHE["geom"]
    X = _prep_x(inputs["x"])
    in_maps = []
    for c in range(CORES):
        m = dict(cm)
        m["x"] = X[c]
        m["masks"] = masks[c]
        in_maps.append(m)
    return in_maps


def _get_runner(nc):
    """Build (once) and cache a jitted shard_map callable for the NEFF.

    run_bass_kernel_spmd re-traces and re-jits on every call; caching the
    callable drops warm-call dispatch to the PJRT execute + transfers.
    """
    if "runner" in _CACHE:
        return _CACHE["runner"]
    import jax
    import numpy as jnp_np  # noqa
    from jax.sharding import Mesh, PartitionSpec
    from jax.experimental.shard_map import shard_map
    import concourse.mybir as mb
    from concourse import bass2jax

    bass2jax.install_neuronx_cc_hook()

    in_names, out_names, out_avals, zero_shapes = [], [], [], []
    partition_name = (
        nc.partition_id_tensor.name if nc.partition_id_tensor else None
    )
    for alloc in nc.m.functions[0].allocations:
        if not isinstance(alloc, mb.MemoryLocationSet):
            continue
        name = alloc.memorylocations[0].name
        if alloc.kind == "ExternalInput":
            if name != partition_name:
                in_names.append(name)
        elif alloc.kind == "ExternalOutput":
            shape = tuple(alloc.tensor_shape)
            dtype = mb.dt.np(alloc.dtype)
            out_names.append(name)
            out_avals.append(jax.core.ShapedArray(shape, dtype))
            zero_shapes.append((shape, dtype))
    n_params = len(in_names)
    all_names = list(in_names) + list(out_names)
    if partition_name is not None:
        all_names.append(partition_name)
    donate = tuple(range(n_params, n_params + len(out_names)))

    def _body(*args):
        operands = list(args)
        if partition_name is not None:
            operands.append(bass2jax.partition_id_tensor())
        outs = bass2jax._bass_exec_p.bind(
            *operands,
            out_avals=tuple(out_avals),
            in_names=tuple(all_names),
            out_names=tuple(out_names),
            lowering_input_output_aliases=(),
            sim_require_finite=True,
            sim_require_nnan=True,
            nc=nc,
        )
        return tuple(outs)

    devices = jax.devices()[:CORES]
    mesh = Mesh(np.asarray(devices), ("core",))
    in_specs = (PartitionSpec("core"),) * (n_params + len(out_names))
    out_specs = (PartitionSpec("core"),) * len(out_names)
    sharded = jax.jit(
        shard_map(_body, mesh=mesh, in_specs=in_specs, out_specs=out_specs,
                  check_rep=False),
        donate_argnums=donate, keep_unused=True,
    )

    zfns = [
        jax.jit(
            lambda s=s, dt=dt: jax.numpy.zeros((CORES * s[0], *s[1:]), dt),
            out_shardings=jax.sharding.NamedSharding(mesh, PartitionSpec("core")),
        )
        for s, dt in zero_shapes
    ]
    in_shard = jax.sharding.NamedSharding(mesh, PartitionSpec("core"))

    def run(concat_in):
        args = []
        for n in in_names:
            v = concat_in[n]
            if isinstance(v, tuple):      # (digest, np array): device-cacheable
                key = ("dev", n, v[0])
                if key not in _CACHE:
                    _CACHE[key] = jax.device_put(v[1], in_shard)
                args.append(_CACHE[key])
            else:
                args.append(v)
        zeros = [zf() for zf in zfns]
        outs = sharded(*args, *zeros)
        return {n: outs[i] for i, n in enumerate(out_names)}

    _CACHE["runner"] = run
    return run


def _weights_digest(inputs):
    import hashlib
    h = hashlib.blake2b(digest_size=16)
    for k in sorted(inputs):
        if k not in ("x",):
            h.update(np.ascontiguousarray(inputs[k]).tobytes())
    return h.hexdigest()


def _make_concat_inputs(inputs):
    """Concatenated-along-core-axis input arrays for the cached runner.
    Weight/mask entries are (digest, array) tuples so the runner can keep
    them device-resident across calls."""
    dig = _weights_digest(inputs)
    cm = _prep_weights(inputs)
    if "geom" not in _CACHE:
        _CACHE["geom"] = _prep_geometry()
    masks = _CACHE["geom"]
    X = _prep_x(inputs["x"])
    cat = {}
    for k, v in cm.items():
        full = np.broadcast_to(v, (CORES,) + v.shape).reshape(
            (CORES * v.shape[0],) + v.shape[1:]
        )
        cat[k] = (dig, full)
    cat["x"] = X.reshape(CORES * G, 128, 2 * T * NP)
    cat["masks"] = ("geom", masks.reshape(CORES * NW, NS * NQ))
    return cat


def kernel(**inputs):
    if "nc" not in _CACHE:
        _CACHE["nc"] = _build_graph()
    nc = _CACHE["nc"]
    run = _get_runner(nc)
    cat = _make_concat_inputs(inputs)
    import time as _time
    t0 = _time.perf_counter()
    outs = run(cat)
    out_np = np.asarray(outs["out"])
    _CACHE["last_device_ns"] = (_time.perf_counter() - t0) * 1e9
    o = out_np.reshape(CORES, NLOC, C).astype(np.float32)
    v = o.reshape(CR, CC_, BR, BC, C).transpose(0, 2, 1, 3, 4)
    return np.ascontiguousarray(v.reshape(N_FULL, 1, C))


# revision 26
# speedup vs baseline: 1.0180x; 1.0180x over previous
"""Trainium2 distributed kernel for nn_AttentionFusion (BEV temporal+spatial attention).

Full computation on device across 8 NeuronCores, zero cross-core communication.

Sharding: 2x4 grid of core blocks (50x25 grid pixels per core). Each core
processes its block PLUS a 2-pixel halo (54x29 local region, out-of-grid
pixels zero) so the spatial neighbor windows are always core-local
(redundant temporal compute on the halo instead of a collective).

Per core:
  phase 1 (13 chunks of 128 px): temporal attention. x arrives channel-major
    (host pre-transposed bf16). h_t is never materialized: tWo is composed
    into the spatial projections on the host; k/v biases cancel or fold
    (softmax shift invariance + sum(p)=1). Writes a local kv table
    [1664, 768] = [k padded to 64/head | v] to DRAM.
  phase 2 (25 chunks of 10x5 queries): spatial window attention. The 126-px
    (14x9) window k is fetched channel-major with one transpose-mode
    dma_gather, v pixel-major with a second gather; scores/ctx are dense
    per-head matmuls, masked by a host-precomputed band mask. exp without
    max-subtraction (scores are tiny); softmax denominator via a ones-column
    matmul fused into the ctx pass.
  phase 3 (10 chunks): output projection.

Self-contained: only needs the container toolchain at /opt/trn_rl_repo.
"""

import math
import os
import sys

import numpy as np

sys.path.insert(0, "/opt/trn_rl_repo")

import ml_dtypes  # noqa: E402

import concourse.bass as bass  # noqa: E402
import concourse.bacc as bacc  # noqa: E402
import concourse.mybir as mybir  # noqa: E402
import concourse.tile as tile  # noqa: E402

F32 = mybir.dt.float32
FP8 = mybir.dt.float8e4
BF16 = mybir.dt.bfloat16
I16 = mybir.dt.int16
AX = mybir.AxisListType
ALU = mybir.AluOpType
ACTF = mybir.ActivationFunctionType

# Problem constants
N_FULL = 10000
GRID = 100
T = 5
C = 256
NH = 8
DK = 32
CORES = 8
CR, CC_ = 2, 4             # core grid 2 x 4
BR, BC = 50, 25            # block rows/cols per core
NLOC = BR * BC             # 1250 real pixels per core
HR, HC = BR + 4, BC + 4    # 54 x 29 local region (with halo)
NH_PIX = HR * HC           # 1566
NP = 128
G = (NH_PIX + NP - 1) // NP        # 13 projection chunks
NPAD = G * NP                      # 1664
GO = 10                            # output-projection chunks
NPO = NLOC // GO                   # 125
# spatial chunks: 10 rows x 5 cols of queries
SQR, SQC = 10, 5
NS_R, NS_C = BR // SQR, BC // SQC  # 5 x 5 = 25
NS = NS_R * NS_C
NQ = SQR * SQC             # 50
WR, WC = SQR + 4, SQC + 4  # 14 x 9
NW = WR * WC               # 126
KROW = 384                 # k section: 3 heads per 128-col group at offsets {0,32,64}
# head slot hh (in kq32 tables) -> standard head: hh = 3*(h%3) + h//3
PERM = [0, 3, 6, 1, 4, 7, 2, 5]
VROW = 256
ROW = KROW + VROW          # 768

_CACHE = {}


def _bf16(a):
    return np.asarray(a, dtype=ml_dtypes.bfloat16)


def _pad_cols(w):
    """[C, 256] -> [C, 384]: head h (32 cols) at 128*(h//3) + 32*(h%3)."""
    w = w.reshape(-1, NH * DK)
    out = np.zeros((w.shape[0], KROW), np.float32)
    for h in range(NH):
        base = 128 * (h // 3) + 32 * (h % 3)
        out[:, base : base + DK] = w[:, DK * h : DK * (h + 1)]
    return out


def _build_graph():
    nc = bacc.Bacc(
        "TRN2",
        target_bir_lowering=False,
        debug=False,
        enable_asserts=False,
        num_devices=CORES,
    )

    # ---------------- I/O ----------------
    x_d = nc.dram_tensor("x", [G, 128, 2 * T * NP], FP8, kind="ExternalInput")
    w_d = {
        "wkv_t": nc.dram_tensor("wkv_t", [2, 128, 2 * C], BF16, kind="ExternalInput"),
        "wq_t": nc.dram_tensor("wq_t", [2, 128, C], BF16, kind="ExternalInput"),
        # spatial q projection, transposed output layout, head-padded [2,128,512]
        "wq_e": nc.dram_tensor("wq_e", [2, 128, KROW], BF16, kind="ExternalInput"),
        # fused (k_padded | v) projection [2, 128, 768]
        "wkv_e": nc.dram_tensor("wkv_e", [2, 128, ROW], BF16, kind="ExternalInput"),
        "wo_s": nc.dram_tensor("wo_s", [2, 128, C], BF16, kind="ExternalInput"),
    }
    b_d = {
        "bq_t": nc.dram_tensor("bq_t", [1, C], BF16, kind="ExternalInput"),
        "bq_e": nc.dram_tensor("bq_e", [1, KROW], BF16, kind="ExternalInput"),
        "bo_e": nc.dram_tensor("bo_e", [1, C], BF16, kind="ExternalInput"),
    }
    ident_d = nc.dram_tensor("ident", [128, 128], BF16, kind="ExternalInput")
    ones1_d = nc.dram_tensor("ones1", [1, 128], BF16, kind="ExternalInput")
    masks_d = nc.dram_tensor("masks", [NW, NS * NQ], BF16, kind="ExternalInput")
    out_d = nc.dram_tensor("out", [NLOC, C], BF16, kind="ExternalOutput")

    with tile.TileContext(nc) as tc:
        with (
            tc.tile_pool(name="const", bufs=1) as cpool,
            tc.tile_pool(name="dram", bufs=1, space="DRAM") as dpool,
            tc.tile_pool(name="sb", bufs=4) as sb,
            tc.tile_pool(name="pkv", bufs=2, space="PSUM") as pkv,
            tc.tile_pool(name="pqt", bufs=1, space="PSUM") as pqt,
            tc.tile_pool(name="pkv1", bufs=1, space="PSUM") as pkv1,
            tc.tile_pool(name="ppv", bufs=1, space="PSUM") as ppv,
            tc.tile_pool(name="ptp", bufs=1, space="PSUM") as ptp,
            tc.tile_pool(name="psc", bufs=1, space="PSUM") as psc,
            tc.tile_pool(name="pcx", bufs=1, space="PSUM") as pcx,
        ):
            v_dram = dpool.tile([NPAD, VROW], BF16, tag="v_dram")

            # ---------- constants ----------
            w_sb = {}
            for n, d in w_d.items():
                t_ = cpool.tile([128, 2, d.shape[2]], BF16, tag=f"w_{n}")
                nc.sync.dma_start(t_[:], d.ap().rearrange("a p c -> p a c"))
                w_sb[n] = t_
            b_sb = {}
            for n, d in b_d.items():
                t_ = cpool.tile([1, d.shape[1]], BF16, tag=f"b_{n}")
                nc.sync.dma_start(t_[:], d.ap())
                b_sb[n] = t_
            ident = cpool.tile([128, 128], BF16, tag="ident")
            nc.sync.dma_start(ident[:], ident_d.ap())
            ones1 = cpool.tile([1, 128], BF16, tag="ones1")
            nc.sync.dma_start(ones1[:], ones1_d.ap())
            onesw = cpool.tile([128, 1], BF16, tag="onesw")
            nc.vector.memset(onesw[:], 1.0)
            masks = cpool.tile([128, NS, NQ], BF16, tag="masks")
            nc.sync.dma_start(
                masks[0:NW, :, :], masks_d.ap().rearrange("w (s q) -> w s q", s=NS)
            )
            kq32 = cpool.tile([32, NH, 2, NH_PIX], BF16, tag="kq32")
            cT_all = cpool.tile([128, 2, NLOC], BF16, tag="cT_all")

            def bias_mm(psum_t, b_key, n_out, rows):
                nc.tensor.matmul(
                    psum_t,
                    ones1[0:1, 0:rows],
                    b_sb[b_key][0:1, 0:n_out],
                    start=False,
                    stop=True,
                )

            # ================= PHASE 1: temporal =================
            for g in range(G):
                xg = sb.tile([128, 2, T, NP], BF16, tag="xg")
                nc.gpsimd.dma_start(
                    xg[:], x_d.ap()[g].rearrange("p (a t n) -> p a t n", a=2, t=T)
                )

                kv_sb = sb.tile([NP, T, 2 * C], BF16, tag="kv_sb")
                for t in range(T):
                    kvp = pkv.tile([NP, 2 * C], F32, tag="kvp")
                    for cc in range(2):
                        nc.tensor.matmul(
                            kvp[:],
                            xg[:, cc, t, :],
                            w_sb["wkv_t"][:, cc, :],
                            start=(cc == 0),
                            stop=(cc == 1),
                        )
                    nc.scalar.copy(kv_sb[:, t, :], kvp[:])
                k_sb = kv_sb[:, :, 0:C]
                v_sb = kv_sb[:, :, C : 2 * C]

                qp = pkv.tile([NP, 2 * C], F32, tag="kvp", name="qp")[:, 0:C]
                for cc in range(2):
                    nc.tensor.matmul(
                        qp[:], xg[:, cc, T - 1, :], w_sb["wq_t"][:, cc, :],
                        start=(cc == 0), stop=False,
                    )
                bias_mm(qp[:], "bq_t", C, NP)
                q_sb = sb.tile([NP, C], BF16, tag="q_sb")
                nc.scalar.copy(q_sb[:], qp[:])

                # scores over t (no max subtraction: |s| < 1)
                prod = sb.tile([NP, T, NH, DK], BF16, tag="prod")
                nc.vector.tensor_mul(
                    prod[:],
                    k_sb.rearrange("p t (h d) -> p t h d", h=NH),
                    q_sb[:].rearrange("p (h d) -> p h d", h=NH)
                    .unsqueeze(1)
                    .broadcast_to((NP, T, NH, DK)),
                )
                s_t = sb.tile([NP, T, NH], BF16, tag="s_t")
                with nc.allow_low_precision(reason="temporal scores bf16"):
                    nc.vector.tensor_reduce(s_t[:], prod[:], axis=AX.X, op=ALU.add)
                es = sb.tile([NP, T, NH], F32, tag="es")
                nc.scalar.activation(es[:], s_t[:], ACTF.Exp)
                tsum = sb.tile([NP, NH], F32, tag="tsum")
                nc.vector.tensor_reduce(
                    tsum[:], es[:].rearrange("p t h -> p h t"), axis=AX.X, op=ALU.add
                )
                rinv = sb.tile([NP, NH], F32, tag="trinv")
                nc.vector.reciprocal(rinv[:], tsum[:])
                p_t = sb.tile([NP, T, NH], BF16, tag="p_t")
                nc.vector.tensor_mul(
                    p_t[:], es[:], rinv[:].unsqueeze(1).broadcast_to((NP, T, NH))
                )

                wv = sb.tile([NP, T, C], BF16, tag="wv")
                nc.vector.tensor_mul(
                    wv[:].rearrange("p t (h d) -> p t h d", h=NH),
                    v_sb.rearrange("p t (h d) -> p t h d", h=NH),
                    p_t[:].unsqueeze(3).broadcast_to((NP, T, NH, DK)),
                )
                c1 = sb.tile([NP, 2, C], BF16, tag="c1")
                nc.vector.tensor_add(c1[:], wv[:, 0:2, :], wv[:, 2:4, :])
                c2 = sb.tile([NP, C], BF16, tag="c2")
                nc.vector.tensor_add(c2[:], c1[:, 0, :], c1[:, 1, :])
                ctx = sb.tile([NP, C], BF16, tag="ctx")
                nc.vector.tensor_add(ctx[:], c2[:], wv[:, 4, :])

                # ctxT (channel-major)
                ctp = ptp.tile([128, 2, NP], BF16, tag="ctp")
                for cc in range(2):
                    nc.tensor.transpose(
                        ctp[:, cc, :], ctx[:, cc * 128 : (cc + 1) * 128],
                        ident[0:NP, 0:NP],
                    )
                ctxT = sb.tile([128, 2, NP], BF16, tag="ctxT")
                nc.scalar.copy(ctxT[:], ctp[:])

                # q'T via transposed projection: psum [128, 4, 128]
                qtp = pqt.tile([128, 3, NP], F32, tag="qtp")
                for grp in range(3):
                    for cc in range(2):
                        nc.tensor.matmul(
                            qtp[:, grp, :],
                            w_sb["wq_e"][:, cc, 128 * grp : 128 * (grp + 1)],
                            ctxT[:, cc, :],
                            start=(cc == 0),
                            stop=False,
                        )
                    nc.tensor.matmul(
                        qtp[:, grp, :],
                        b_sb["bq_e"][0:1, 128 * grp : 128 * (grp + 1)],
                        ones1[0:1, 0:NP],
                        start=False,
                        stop=True,
                    )
                kq_tmp = sb.tile([128, 3, 2, NP], BF16, tag="kq_tmp")
                nc.vector.tensor_copy(kq_tmp[:, :, 1, :], qtp[:])

                # k1 transposed projection
                ktp = pkv1.tile([128, 3, NP], F32, tag="kv1")
                for grp in range(3):
                    for cc in range(2):
                        nc.tensor.matmul(
                            ktp[:, grp, :],
                            w_sb["wkv_e"][:, cc, 128 * grp : 128 * (grp + 1)],
                            ctxT[:, cc, :],
                            start=(cc == 0),
                            stop=(cc == 1),
                        )
                nc.scalar.copy(kq_tmp[:, :, 0, :], ktp[:])
                # head regroup: partitions [32m:32m+32) -> head slots [3m:3m+3)
                glo = g * NP
                hi = min(NH_PIX - glo, NP)
                if hi > 0:
                    for m in range(3):
                        nc.sync.dma_start(
                            kq32[:, 3 * m : min(3 * m + 3, NH), :, glo : glo + hi],
                            kq_tmp[32 * m : 32 * m + 32, 0 : (3 if m < 2 else 2), :, 0:hi],
                        )
                vp = ppv.tile([NP, VROW], F32, tag="vp")
                for cc in range(2):
                    nc.tensor.matmul(
                        vp[:], ctxT[:, cc, :], w_sb["wkv_e"][:, cc, KROW:ROW],
                        start=(cc == 0), stop=(cc == 1),
                    )
                v1_sb = sb.tile([NP, VROW], BF16, tag="v1_sb")
                nc.scalar.copy(v1_sb[:], vp[:])
                nc.gpsimd.dma_start(
                    v_dram[g * NP : (g + 1) * NP, :], v1_sb[:]
                )

            PH = os.environ.get("KERNEL_PHASES", "123")
            # ================= PHASE 2: spatial =================
            for s in (range(NS) if "2" in PH else []):
                r0, c0 = (s // NS_C) * SQR, (s % NS_C) * SQC
                vw = sb.tile([NW, VROW], BF16, tag="vw")
                nc.gpsimd.dma_start(
                    vw[:],
                    v_dram[0:NH_PIX, :].rearrange("(r c) x -> r c x", r=HR)[
                        r0 : r0 + WR, c0 : c0 + WC, :
                    ],
                )
                kqv = kq32[:].rearrange("p h t (r c) -> p h t r c", r=HR)
                kTw = sb.tile([32, NH, NW], BF16, tag="kTw")
                nc.vector.tensor_copy(
                    kTw[:].rearrange("p h (r c) -> p h r c", r=WR),
                    kqv[:, :, 0, r0 : r0 + WR, c0 : c0 + WC],
                )
                qw = sb.tile([32, NH, NQ], BF16, tag="qw")
                nc.vector.tensor_copy(
                    qw[:].rearrange("p h (r c) -> p h r c", r=SQR),
                    kqv[:, :, 1, 2 + r0 : 2 + r0 + SQR, 2 + c0 : 2 + c0 + SQC],
                )

                LVL = int(os.environ.get("KERNEL_P2LVL", "9"))
                if LVL < 1:
                    continue
                sc = psc.tile([NW, NH, NQ], F32, tag="sc")
                for h in range(NH):
                    nc.tensor.matmul(
                        sc[:, h, :],
                        kTw[:, h, :],
                        qw[:, h, :],
                        start=True,
                        stop=True,
                    )
                if LVL < 2:
                    continue
                E = sb.tile([NW, NH, NQ], BF16, tag="E")
                nc.scalar.activation(E[:], sc[:], ACTF.Exp)
                E2 = sb.tile([NW, NH, NQ], BF16, tag="E2")
                nc.gpsimd.tensor_mul(
                    E2[:],
                    E[:],
                    masks[0:NW, s, :].unsqueeze(1).broadcast_to((NW, NH, NQ)),
                )

                if LVL < 3:
                    continue
                cx = pcx.tile([NQ, NH, DK + 1], F32, tag="cx")
                for h in range(NH):
                    nc.tensor.matmul(
                        cx[:, h, 0:DK],
                        E2[:, h, :],
                        vw[0:NW, DK * PERM[h] : DK * PERM[h] + DK],
                        start=True,
                        stop=True,
                    )
                    nc.tensor.matmul(
                        cx[:, h, DK : DK + 1],
                        E2[:, h, :],
                        onesw[0:NW, :],
                        start=True,
                        stop=True,
                    )
                srinv = sb.tile([NQ, NH], F32, tag="srinv")
                nc.vector.reciprocal(srinv[:], cx[:, :, DK])
                ctxn = sb.tile([NQ, C], BF16, tag="ctxn")
                nc.vector.tensor_mul(
                    ctxn[:].rearrange("q (h d) -> q h d", h=NH),
                    cx[:, :, 0:DK],
                    srinv[:].unsqueeze(2).broadcast_to((NQ, NH, DK)),
                )
                if LVL < 4:
                    continue
                ntp = ptp.tile([128, 2, NQ], BF16, tag="ctp")
                for cc in range(2):
                    nc.tensor.transpose(
                        ntp[:, cc, :], ctxn[:, cc * 128 : (cc + 1) * 128],
                        ident[0:NQ, 0:NQ],
                    )
                csel = cT_all[:, :, :].rearrange(
                    "p a (r c) -> p a r c", r=BR
                )[:, :, r0 : r0 + SQR, c0 : c0 + SQC]
                nc.scalar.copy(
                    csel[:],
                    ntp[:].rearrange("p a (r c) -> p a r c", r=SQR),
                )

            # ================= PHASE 3: output proj =================
            if "3" not in PH:
                zz = sb.tile([NPO, C], BF16, tag="o_sb", name="zz")
                nc.vector.memset(zz[:], 0.0)
                nc.gpsimd.dma_start(out_d.ap()[0:NPO, :], zz[:])
            for g in (range(GO) if "3" in PH else []):
                op = pkv.tile([NPO, 2 * C], F32, tag="kvp", name="op")[:, 0:C]
                for cc in range(2):
                    nc.tensor.matmul(
                        op[:], cT_all[:, cc, g * NPO : (g + 1) * NPO],
                        w_sb["wo_s"][:, cc, :],
                        start=(cc == 0), stop=False,
                    )
                bias_mm(op[:], "bo_e", C, NPO)
                o_sb = sb.tile([NPO, C], BF16, tag="o_sb")
                nc.scalar.copy(o_sb[:], op[:])
                nc.gpsimd.dma_start(out_d.ap()[g * NPO : (g + 1) * NPO, :], o_sb[:])

    nc.compile()
    return nc


def _prep_weights(inputs):
    """Host-side weight transforms (all small)."""
    scale = 1.0 / math.sqrt(DK)
    f = lambda k: np.asarray(inputs[k], np.float32)
    tWq, tbq = f("t_Wq") * scale, f("t_bq") * scale
    tWk = f("t_Wk")
    tWv, tbv = f("t_Wv"), f("t_bv")
    tWo, tbo = f("t_Wo"), f("t_bo")
    sWq, sbq = f("s_Wq"), f("s_bq")
    sWk = f("s_Wk")
    sWv, sbv = f("s_Wv"), f("s_bv")
    sWo, sbo = f("s_Wo"), f("s_bo")

    hb = tbv @ tWo + tbo                    # constant part of h_t
    Wq_eff = (tWo @ sWq) * scale
    bq_eff = (hb @ sWq + sbq) * scale
    Wk_eff = tWo @ sWk                      # k bias dropped (softmax-invariant)
    Wv_eff = tWo @ sWv
    cv = hb @ sWv + sbv                     # constant part of v1
    bo_eff = sbo + cv @ sWo

    cm = {
        "wkv_t": _bf16(np.concatenate([tWk, tWv], axis=1).reshape(2, 128, 2 * C)),
        "wq_t": _bf16(tWq.reshape(2, 128, C)),
        "wq_e": _bf16(_pad_cols(Wq_eff).reshape(2, 128, KROW)),
        "wkv_e": _bf16(
            np.concatenate([_pad_cols(Wk_eff), Wv_eff], axis=1).reshape(2, 128, ROW)
        ),
        "wo_s": _bf16(
            sWo.reshape(NH, DK, C)[PERM].reshape(2, 128, C)
        ),
        "bq_t": _bf16(tbq.reshape(1, C)),
        "bq_e": _bf16(_pad_cols(bq_eff).reshape(1, KROW)),
        "bo_e": _bf16(bo_eff.reshape(1, C)),
        "ident": _bf16(np.eye(128, dtype=np.float32)),
        "ones1": _bf16(np.ones((1, 128), np.float32)),
    }
    return cm


def _prep_geometry():
    """Per-core gather indices (local coords) and window masks."""
    masks = np.zeros((CORES, NW, NS * NQ), np.float32)
    for R in range(CR):
        for C4 in range(CC_):
            core = R * CC_ + C4
            for s in range(NS):
                r0, c0 = (s // NS_C) * SQR, (s % NS_C) * SQC
                gr0, gc0 = BR * R + r0, BC * C4 + c0
                wr = gr0 - 2 + np.arange(WR)          # global window rows
                wc = gc0 - 2 + np.arange(WC)
                valid = (wr[:, None] >= 0) & (wr[:, None] < GRID) & \
                        (wc[None, :] >= 0) & (wc[None, :] < GRID)
                qr = gr0 + np.arange(SQR)
                qc = gc0 + np.arange(SQC)
                qrc = np.clip(qr, 2, GRID - 3)
                qcc = np.clip(qc, 2, GRID - 3)
                mrow = (np.abs(wr[:, None] - qrc[None, :]) <= 2)
                mcol = (np.abs(wc[:, None] - qcc[None, :]) <= 2)
                m = (mrow[:, None, :, None] & mcol[None, :, None, :] &
                     valid[:, :, None, None])
                masks[core, :, s * NQ : (s + 1) * NQ] = m.reshape(NW, NQ)
    return _bf16(masks)


def _prep_x(x):
    """x [10000, 5, 256] f32 -> per-core halo-extended channel-major bf16
    chunks [8, G, 128, 2*T*128]."""
    xb = np.asarray(x, np.float32).astype(ml_dtypes.float8_e4m3).reshape(
        GRID, GRID, T, C
    )
    xp = np.zeros((GRID + 4, GRID + 4, T, C), dtype=xb.dtype)
    xp[2 : 2 + GRID, 2 : 2 + GRID] = xb
    out = np.zeros((CORES, G, 128, 2 * T * NP), dtype=xb.dtype)
    for R in range(CR):
        for C4 in range(CC_):
            core = R * CC_ + C4
            blk = xp[BR * R : BR * R + HR, BC * C4 : BC * C4 + HC]  # [54,29,T,C]
            flat = blk.reshape(NH_PIX, T, C)
            flat = np.concatenate(
                [flat, np.zeros((NPAD - NH_PIX, T, C), dtype=xb.dtype)], axis=0
            )
            v = flat.reshape(G, NP, T, 2, 128)
            v = v.transpose(0, 4, 3, 2, 1)      # (g, ch, cc, t, px)
            out[core] = v.reshape(G, 128, 2 * T * NP)
    return out


def _unprep_out(res_list):
    """[8][1250, 256] bf16 -> [10000, 1, 256] f32 global row-major."""
    o = np.stack([np.asarray(r) for r in res_list], axis=0).astype(np.float32)
    v = o.reshape(CR, CC_, BR, BC, C)
    v = v.transpose(0, 2, 1, 3, 4)
    return np.ascontiguousarray(v.reshape(N_FULL, 1, C))


def _make_in_maps(inputs):
    cm = _prep_weights(inputs)
    if "geom" not in _CACHE:
        _CACHE["geom"] = _prep_geometry()
    masks = _CACHE["geom"]
    X = _prep_x(inputs["x"])
    in_maps = []
    for c in range(CORES):
        m = dict(cm)
        m["x"] = X[c]
        m["masks"] = masks[c]
        in_maps.append(m)
    return in_maps


def _get_runner(nc):
    """Build (once) and cache a jitted shard_map callable for the NEFF.

    run_bass_kernel_spmd re-traces and re-jits on every call; caching the
    callable drops warm-call dispatch to the PJRT execute + transfers.
    """
    if "runner" in _CACHE:
        return _CACHE["runner"]
    import jax
    import numpy as jnp_np  # noqa
    from jax.sharding import Mesh, PartitionSpec
    from jax.experimental.shard_map import shard_map
    import concourse.mybir as mb
    from concourse import bass2jax

    bass2jax.install_neuronx_cc_hook()

    in_names, out_names, out_avals, zero_shapes = [], [], [], []
    partition_name = (
        nc.partition_id_tensor.name if nc.partition_id_tensor else None
    )
    for alloc in nc.m.functions[0].allocations:
        if not isinstance(alloc, mb.MemoryLocationSet):
            continue
        name = alloc.memorylocations[0].name
        if alloc.kind == "ExternalInput":
            if name != partition_name:
                in_names.append(name)
        elif alloc.kind == "ExternalOutput":
            shape = tuple(alloc.tensor_shape)
            dtype = mb.dt.np(alloc.dtype)
            out_names.append(name)
            out_avals.append(jax.core.ShapedArray(shape, dtype))
            zero_shapes.append((shape, dtype))
    n_params = len(in_names)
    all_names = list(in_names) + list(out_names)
    if partition_name is not None:
        all_names.append(partition_name)
    donate = tuple(range(n_params, n_params + len(out_names)))

    def _body(*args):
        operands = list(args)
        if partition_name is not None:
            operands.append(bass2jax.partition_id_tensor())
        outs = bass2jax._bass_exec_p.bind(
            *operands,
            out_avals=tuple(out_avals),
            in_names=tuple(all_names),
            out_names=tuple(out_names),
            lowering_input_output_aliases=(),
            sim_require_finite=True,
            sim_require_nnan=True,
            nc=nc,
        )
        return tuple(outs)

    devices = jax.devices()[:CORES]
    mesh = Mesh(np.asarray(devices), ("core",))
    in_specs = (PartitionSpec("core"),) * (n_params + len(out_names))
    out_specs = (PartitionSpec("core"),) * len(out_names)
    sharded = jax.jit(
        shard_map(_body, mesh=mesh, in_specs=in_specs, out_specs=out_specs,
                  check_rep=False),
        donate_argnums=donate, keep_unused=True,
    )

    zfns = [
        jax.jit(
            lambda s=s, dt=dt: jax.numpy.zeros((CORES * s[0], *s[1:]), dt),
            out_shardings=jax.sharding.NamedSharding(mesh, PartitionSpec("core")),
        )
        for s, dt in zero_shapes
    ]
    in_shard = jax.sharding.NamedSharding(mesh, PartitionSpec("core"))

    def run(concat_in):
        args = []
        for n in in_names:
            v = concat_in[n]
            if isinstance(v, tuple):      # (digest, np array): device-cacheable
                key = ("dev", n, v[0])
                if key not in _CACHE:
                    _CACHE[key] = jax.device_put(v[1], in_shard)
                args.append(_CACHE[key])
            else:
                args.append(v)
        zeros = [zf() for zf in zfns]
        outs = sharded(*args, *zeros)
        return {n: outs[i] for i, n in enumerate(out_names)}

    _CACHE["runner"] = run
    return run


def _weights_digest(inputs):
    import hashlib
    h = hashlib.blake2b(digest_size=16)
    for k in sorted(inputs):
        if k not in ("x",):
            h.update(np.ascontiguousarray(inputs[k]).tobytes())
    return h.hexdigest()


def _make_concat_inputs(inputs):
    """Concatenated-along-core-axis input arrays for the cached runner.
    Weight/mask entries are (digest, array) tuples so the runner can keep
    them device-resident across calls."""
    dig = _weights_digest(inputs)
    cm = _prep_weights(inputs)
    if "geom" not in _CACHE:
        _CACHE["geom"] = _prep_geometry()
    masks = _CACHE["geom"]
    X = _prep_x(inputs["x"])
    cat = {}
    for k, v in cm.items():
        full = np.broadcast_to(v, (CORES,) + v.shape).reshape(
            (CORES * v.shape[0],) + v.shape[1:]
        )
        cat[k] = (dig, full)
    cat["x"] = X.reshape(CORES * G, 128, 2 * T * NP)
    cat["masks"] = ("geom", masks.reshape(CORES * NW, NS * NQ))
    return cat


def kernel(**inputs):
    if "nc" not in _CACHE:
        _CACHE["nc"] = _build_graph()
    nc = _CACHE["nc"]
    run = _get_runner(nc)
    cat = _make_concat_inputs(inputs)
    import time as _time
    t0 = _time.perf_counter()
    outs = run(cat)
    out_np = np.asarray(outs["out"])
    _CACHE["last_device_ns"] = (_time.perf_counter() - t0) * 1e9
    o = out_np.reshape(CORES, NLOC, C).astype(np.float32)
    v = o.reshape(CR, CC_, BR, BC, C).transpose(0, 2, 1, 3, 4)
    return np.ascontiguousarray(v.reshape(N_FULL, 1, C))


# revision 30
# speedup vs baseline: 1.0199x; 1.0019x over previous
"""Trainium2 distributed kernel for nn_AttentionFusion (BEV temporal+spatial attention).

Full computation on device across 8 NeuronCores, zero cross-core communication.

Sharding: 2x4 grid of core blocks (50x25 grid pixels per core). Each core
processes its block PLUS a 2-pixel halo (54x29 local region, out-of-grid
pixels zero) so the spatial neighbor windows are always core-local
(redundant temporal compute on the halo instead of a collective).

Per core:
  phase 1 (13 chunks of 128 px): temporal attention. x arrives channel-major
    (host pre-transposed bf16). h_t is never materialized: tWo is composed
    into the spatial projections on the host; k/v biases cancel or fold
    (softmax shift invariance + sum(p)=1). Writes a local kv table
    [1664, 768] = [k padded to 64/head | v] to DRAM.
  phase 2 (25 chunks of 10x5 queries): spatial window attention. The 126-px
    (14x9) window k is fetched channel-major with one transpose-mode
    dma_gather, v pixel-major with a second gather; scores/ctx are dense
    per-head matmuls, masked by a host-precomputed band mask. exp without
    max-subtraction (scores are tiny); softmax denominator via a ones-column
    matmul fused into the ctx pass.
  phase 3 (10 chunks): output projection.

Self-contained: only needs the container toolchain at /opt/trn_rl_repo.
"""

import math
import os
import sys

import numpy as np

sys.path.insert(0, "/opt/trn_rl_repo")

import ml_dtypes  # noqa: E402

import concourse.bass as bass  # noqa: E402
import concourse.bacc as bacc  # noqa: E402
import concourse.mybir as mybir  # noqa: E402
import concourse.tile as tile  # noqa: E402

F32 = mybir.dt.float32
FP8 = mybir.dt.float8e4
BF16 = mybir.dt.bfloat16
I16 = mybir.dt.int16
AX = mybir.AxisListType
ALU = mybir.AluOpType
ACTF = mybir.ActivationFunctionType

# Problem constants
N_FULL = 10000
GRID = 100
T = 5
C = 256
NH = 8
DK = 32
CORES = 8
CR, CC_ = 2, 4             # core grid 2 x 4
BR, BC = 50, 25            # block rows/cols per core
NLOC = BR * BC             # 1250 real pixels per core
HR, HC = BR + 4, BC + 4    # 54 x 29 local region (with halo)
NH_PIX = HR * HC           # 1566
NP = 128
G = (NH_PIX + NP - 1) // NP        # 13 projection chunks
NPAD = G * NP                      # 1664
GO = 10                            # output-projection chunks
NPO = NLOC // GO                   # 125
# spatial chunks: 10 rows x 5 cols of queries
SQR, SQC = 10, 5
NS_R, NS_C = BR // SQR, BC // SQC  # 5 x 5 = 25
NS = NS_R * NS_C
NQ = SQR * SQC             # 50
WR, WC = SQR + 4, SQC + 4  # 14 x 9
NW = WR * WC               # 126
KROW = 384                 # k section: 3 heads per 128-col group at offsets {0,32,64}
# head slot hh (in kq32 tables) -> standard head: hh = 3*(h%3) + h//3
PERM = [0, 3, 6, 1, 4, 7, 2, 5]
VROW = 256
ROW = KROW + VROW          # 768
SEGR = 34                  # rows per overlap segment (2 segments: rows [0,34), [20,54))
SEGP = SEGR * HC           # 986 pixels per segment

_CACHE = {}


def _bf16(a):
    return np.asarray(a, dtype=ml_dtypes.bfloat16)


def _pad_cols(w):
    """[C, 256] -> [C, 384]: head h (32 cols) at 128*(h//3) + 32*(h%3)."""
    w = w.reshape(-1, NH * DK)
    out = np.zeros((w.shape[0], KROW), np.float32)
    for h in range(NH):
        base = 128 * (h // 3) + 32 * (h % 3)
        out[:, base : base + DK] = w[:, DK * h : DK * (h + 1)]
    return out


def _build_graph():
    nc = bacc.Bacc(
        "TRN2",
        target_bir_lowering=False,
        debug=False,
        enable_asserts=False,
        num_devices=CORES,
    )

    # ---------------- I/O ----------------
    x_d = nc.dram_tensor("x", [G, 128, 2 * T * NP], FP8, kind="ExternalInput")
    w_d = {
        "wkv_t": nc.dram_tensor("wkv_t", [2, 128, 2 * C], BF16, kind="ExternalInput"),
        "wq_t": nc.dram_tensor("wq_t", [2, 128, C], BF16, kind="ExternalInput"),
        # spatial q projection, transposed output layout, head-padded [2,128,512]
        "wq_e": nc.dram_tensor("wq_e", [2, 128, KROW], BF16, kind="ExternalInput"),
        # fused (k_padded | v) projection [2, 128, 768]
        "wkv_e": nc.dram_tensor("wkv_e", [2, 128, ROW], BF16, kind="ExternalInput"),
        "wo_s": nc.dram_tensor("wo_s", [2, 128, C], BF16, kind="ExternalInput"),
    }
    b_d = {
        "bq_t": nc.dram_tensor("bq_t", [1, C], BF16, kind="ExternalInput"),
        "bq_e": nc.dram_tensor("bq_e", [1, KROW], BF16, kind="ExternalInput"),
        "bo_e": nc.dram_tensor("bo_e", [1, C], BF16, kind="ExternalInput"),
    }
    ident_d = nc.dram_tensor("ident", [128, 128], BF16, kind="ExternalInput")
    ones1_d = nc.dram_tensor("ones1", [1, 128], BF16, kind="ExternalInput")
    masks_d = nc.dram_tensor("masks", [NW, NS * NQ], BF16, kind="ExternalInput")
    out_d = nc.dram_tensor("out", [NLOC, C], BF16, kind="ExternalOutput")

    with tile.TileContext(nc) as tc:
        with (
            tc.tile_pool(name="const", bufs=1) as cpool,
            tc.tile_pool(name="dram", bufs=1, space="DRAM") as dpool,
            tc.tile_pool(name="sb", bufs=4) as sb,
            tc.tile_pool(name="pkv", bufs=2, space="PSUM") as pkv,
            tc.tile_pool(name="pqt", bufs=1, space="PSUM") as pqt,
            tc.tile_pool(name="pkv1", bufs=1, space="PSUM") as pkv1,
            tc.tile_pool(name="ppv", bufs=1, space="PSUM") as ppv,
            tc.tile_pool(name="ptp", bufs=1, space="PSUM") as ptp,
            tc.tile_pool(name="psc", bufs=1, space="PSUM") as psc,
            tc.tile_pool(name="pcx", bufs=1, space="PSUM") as pcx,
        ):
            v_dram = [
                dpool.tile([SEGP, VROW], BF16, tag=f"v_dram{i}", name=f"v_dram{i}")
                for i in range(2)
            ]

            # ---------- constants ----------
            w_sb = {}
            for n, d in w_d.items():
                t_ = cpool.tile([128, 2, d.shape[2]], BF16, tag=f"w_{n}")
                nc.sync.dma_start(t_[:], d.ap().rearrange("a p c -> p a c"))
                w_sb[n] = t_
            b_sb = {}
            for n, d in b_d.items():
                t_ = cpool.tile([1, d.shape[1]], BF16, tag=f"b_{n}")
                nc.sync.dma_start(t_[:], d.ap())
                b_sb[n] = t_
            ident = cpool.tile([128, 128], BF16, tag="ident")
            nc.sync.dma_start(ident[:], ident_d.ap())
            ones1 = cpool.tile([1, 128], BF16, tag="ones1")
            nc.sync.dma_start(ones1[:], ones1_d.ap())
            onesw = cpool.tile([128, 1], BF16, tag="onesw")
            nc.vector.memset(onesw[:], 1.0)
            masks = cpool.tile([128, NS, NQ], BF16, tag="masks")
            nc.sync.dma_start(
                masks[0:NW, :, :], masks_d.ap().rearrange("w (s q) -> w s q", s=NS)
            )
            kq32 = [
                cpool.tile([32, NH, 2, SEGP], BF16, tag=f"kq32_{i}", name=f"kq32_{i}")
                for i in range(2)
            ]
            cT_all = cpool.tile([128, 2, NLOC], BF16, tag="cT_all")

            def bias_mm(psum_t, b_key, n_out, rows):
                nc.tensor.matmul(
                    psum_t,
                    ones1[0:1, 0:rows],
                    b_sb[b_key][0:1, 0:n_out],
                    start=False,
                    stop=True,
                )

            # ================= PHASE 1: temporal =================
            for g in range(G):
                xg = sb.tile([128, 2, T, NP], BF16, tag="xg")
                nc.gpsimd.dma_start(
                    xg[:], x_d.ap()[g].rearrange("p (a t n) -> p a t n", a=2, t=T)
                )

                kv_sb = sb.tile([NP, T, 2 * C], BF16, tag="kv_sb")
                for t in range(T):
                    kvp = pkv.tile([NP, 2 * C], F32, tag="kvp")
                    for cc in range(2):
                        nc.tensor.matmul(
                            kvp[:],
                            xg[:, cc, t, :],
                            w_sb["wkv_t"][:, cc, :],
                            start=(cc == 0),
                            stop=(cc == 1),
                        )
                    nc.scalar.copy(kv_sb[:, t, :], kvp[:])
                k_sb = kv_sb[:, :, 0:C]
                v_sb = kv_sb[:, :, C : 2 * C]

                qp = pkv.tile([NP, 2 * C], F32, tag="kvp", name="qp")[:, 0:C]
                for cc in range(2):
                    nc.tensor.matmul(
                        qp[:], xg[:, cc, T - 1, :], w_sb["wq_t"][:, cc, :],
                        start=(cc == 0), stop=False,
                    )
                bias_mm(qp[:], "bq_t", C, NP)
                q_sb = sb.tile([NP, C], BF16, tag="q_sb")
                nc.scalar.copy(q_sb[:], qp[:])

                # scores over t (no max subtraction: |s| < 1)
                prod = sb.tile([NP, T, NH, DK], BF16, tag="prod")
                nc.vector.tensor_mul(
                    prod[:],
                    k_sb.rearrange("p t (h d) -> p t h d", h=NH),
                    q_sb[:].rearrange("p (h d) -> p h d", h=NH)
                    .unsqueeze(1)
                    .broadcast_to((NP, T, NH, DK)),
                )
                s_t = sb.tile([NP, T, NH], BF16, tag="s_t")
                with nc.allow_low_precision(reason="temporal scores bf16"):
                    nc.vector.tensor_reduce(s_t[:], prod[:], axis=AX.X, op=ALU.add)
                es = sb.tile([NP, T, NH], F32, tag="es")
                nc.scalar.activation(es[:], s_t[:], ACTF.Exp)
                tsum = sb.tile([NP, NH], F32, tag="tsum")
                nc.vector.tensor_reduce(
                    tsum[:], es[:].rearrange("p t h -> p h t"), axis=AX.X, op=ALU.add
                )
                rinv = sb.tile([NP, NH], F32, tag="trinv")
                nc.vector.reciprocal(rinv[:], tsum[:])
                p_t = sb.tile([NP, T, NH], BF16, tag="p_t")
                nc.vector.tensor_mul(
                    p_t[:], es[:], rinv[:].unsqueeze(1).broadcast_to((NP, T, NH))
                )

                wv = sb.tile([NP, T, C], BF16, tag="wv")
                nc.vector.tensor_mul(
                    wv[:].rearrange("p t (h d) -> p t h d", h=NH),
                    v_sb.rearrange("p t (h d) -> p t h d", h=NH),
                    p_t[:].unsqueeze(3).broadcast_to((NP, T, NH, DK)),
                )
                c1 = sb.tile([NP, 2, C], BF16, tag="c1")
                nc.vector.tensor_add(c1[:], wv[:, 0:2, :], wv[:, 2:4, :])
                c2 = sb.tile([NP, C], BF16, tag="c2")
                nc.vector.tensor_add(c2[:], c1[:, 0, :], c1[:, 1, :])
                ctx = sb.tile([NP, C], BF16, tag="ctx")
                nc.vector.tensor_add(ctx[:], c2[:], wv[:, 4, :])

                # ctxT (channel-major)
                ctp = ptp.tile([128, 2, NP], BF16, tag="ctp")
                for cc in range(2):
                    nc.tensor.transpose(
                        ctp[:, cc, :], ctx[:, cc * 128 : (cc + 1) * 128],
                        ident[0:NP, 0:NP],
                    )
                ctxT = sb.tile([128, 2, NP], BF16, tag="ctxT")
                nc.scalar.copy(ctxT[:], ctp[:])

                # q'T via transposed projection: psum [128, 4, 128]
                qtp = pqt.tile([128, 3, NP], F32, tag="qtp")
                for grp in range(3):
                    for cc in range(2):
                        nc.tensor.matmul(
                            qtp[:, grp, :],
                            w_sb["wq_e"][:, cc, 128 * grp : 128 * (grp + 1)],
                            ctxT[:, cc, :],
                            start=(cc == 0),
                            stop=False,
                        )
                    nc.tensor.matmul(
                        qtp[:, grp, :],
                        b_sb["bq_e"][0:1, 128 * grp : 128 * (grp + 1)],
                        ones1[0:1, 0:NP],
                        start=False,
                        stop=True,
                    )
                kq_tmp = sb.tile([128, 3, 2, NP], BF16, tag="kq_tmp")
                nc.vector.tensor_copy(kq_tmp[:, :, 1, :], qtp[:])

                # k1 transposed projection
                ktp = pkv1.tile([128, 3, NP], F32, tag="kv1")
                for grp in range(3):
                    for cc in range(2):
                        nc.tensor.matmul(
                            ktp[:, grp, :],
                            w_sb["wkv_e"][:, cc, 128 * grp : 128 * (grp + 1)],
                            ctxT[:, cc, :],
                            start=(cc == 0),
                            stop=(cc == 1),
                        )
                nc.scalar.copy(kq_tmp[:, :, 0, :], ktp[:])
                # head regroup: partitions [32m:32m+32) -> head slots [3m:3m+3),
                # scattered into the overlap segments for phase-1/2 pipelining
                glo, ghi = g * NP, min(NH_PIX, (g + 1) * NP)
                for seg in range(2):
                    slo = seg * 20 * HC
                    shi = slo + SEGP
                    lo, hi = max(glo, slo), min(ghi, shi)
                    if lo >= hi:
                        continue
                    for m in range(3):
                        nc.sync.dma_start(
                            kq32[seg][:, 3 * m : min(3 * m + 3, NH), :,
                                      lo - slo : hi - slo],
                            kq_tmp[32 * m : 32 * m + 32,
                                   0 : (3 if m < 2 else 2), :,
                                   lo - glo : hi - glo],
                        )
                vp = ppv.tile([NP, VROW], F32, tag="vp")
                for cc in range(2):
                    nc.tensor.matmul(
                        vp[:], ctxT[:, cc, :], w_sb["wkv_e"][:, cc, KROW:ROW],
                        start=(cc == 0), stop=(cc == 1),
                    )
                v1_sb = sb.tile([NP, VROW], BF16, tag="v1_sb")
                nc.scalar.copy(v1_sb[:], vp[:])
                for seg in range(2):
                    slo = seg * 20 * HC
                    shi = slo + SEGP
                    lo, hi = max(glo, slo), min(ghi, shi)
                    if lo < hi:
                        nc.gpsimd.dma_start(
                            v_dram[seg][lo - slo : hi - slo, :],
                            v1_sb[lo - glo : hi - glo, :],
                        )

            PH = os.environ.get("KERNEL_PHASES", "123")
            # ================= PHASE 2: spatial =================
            for s in (range(NS) if "2" in PH else []):
                r0, c0 = (s // NS_C) * SQR, (s % NS_C) * SQC
                seg = 0 if r0 <= 20 else 1
                rs = r0 - 20 * seg
                vw = sb.tile([NW, VROW], BF16, tag="vw")
                nc.gpsimd.dma_start(
                    vw[:],
                    v_dram[seg][:].rearrange("(r c) x -> r c x", r=SEGR)[
                        rs : rs + WR, c0 : c0 + WC, :
                    ],
                )
                kqv = kq32[seg][:].rearrange("p h t (r c) -> p h t r c", r=SEGR)
                kTw = sb.tile([32, NH, NW], BF16, tag="kTw")
                nc.vector.tensor_copy(
                    kTw[:].rearrange("p h (r c) -> p h r c", r=WR),
                    kqv[:, :, 0, rs : rs + WR, c0 : c0 + WC],
                )
                qw = sb.tile([32, NH, NQ], BF16, tag="qw")
                nc.vector.tensor_copy(
                    qw[:].rearrange("p h (r c) -> p h r c", r=SQR),
                    kqv[:, :, 1, 2 + rs : 2 + rs + SQR, 2 + c0 : 2 + c0 + SQC],
                )

                LVL = int(os.environ.get("KERNEL_P2LVL", "9"))
                if LVL < 1:
                    continue
                sc = psc.tile([NW, NH, NQ], F32, tag="sc")
                for h in range(NH):
                    nc.tensor.matmul(
                        sc[:, h, :],
                        kTw[:, h, :],
                        qw[:, h, :],
                        start=True,
                        stop=True,
                    )
                if LVL < 2:
                    continue
                E = sb.tile([NW, NH, NQ], BF16, tag="E")
                nc.scalar.activation(E[:], sc[:], ACTF.Exp)
                E2 = sb.tile([NW, NH, NQ], BF16, tag="E2")
                nc.gpsimd.tensor_mul(
                    E2[:],
                    E[:],
                    masks[0:NW, s, :].unsqueeze(1).broadcast_to((NW, NH, NQ)),
                )

                if LVL < 3:
                    continue
                cx = pcx.tile([NQ, NH, DK + 1], F32, tag="cx")
                for h in range(NH):
                    nc.tensor.matmul(
                        cx[:, h, 0:DK],
                        E2[:, h, :],
                        vw[0:NW, DK * PERM[h] : DK * PERM[h] + DK],
                        start=True,
                        stop=True,
                    )
                    nc.tensor.matmul(
                        cx[:, h, DK : DK + 1],
                        E2[:, h, :],
                        onesw[0:NW, :],
                        start=True,
                        stop=True,
                    )
                srinv = sb.tile([NQ, NH], F32, tag="srinv")
                nc.vector.reciprocal(srinv[:], cx[:, :, DK])
                ctxn = sb.tile([NQ, C], BF16, tag="ctxn")
                nc.vector.tensor_mul(
                    ctxn[:].rearrange("q (h d) -> q h d", h=NH),
                    cx[:, :, 0:DK],
                    srinv[:].unsqueeze(2).broadcast_to((NQ, NH, DK)),
                )
                if LVL < 4:
                    continue
                ntp = ptp.tile([128, 2, NQ], BF16, tag="ctp")
                for cc in range(2):
                    nc.tensor.transpose(
                        ntp[:, cc, :], ctxn[:, cc * 128 : (cc + 1) * 128],
                        ident[0:NQ, 0:NQ],
                    )
                csel = cT_all[:, :, :].rearrange(
                    "p a (r c) -> p a r c", r=BR
                )[:, :, r0 : r0 + SQR, c0 : c0 + SQC]
                nc.scalar.copy(
                    csel[:],
                    ntp[:].rearrange("p a (r c) -> p a r c", r=SQR),
                )

            # ================= PHASE 3: output proj =================
            if "3" not in PH:
                zz = sb.tile([NPO, C], BF16, tag="o_sb", name="zz")
                nc.vector.memset(zz[:], 0.0)
                nc.gpsimd.dma_start(out_d.ap()[0:NPO, :], zz[:])
            for g in (range(GO) if "3" in PH else []):
                op = pkv.tile([NPO, 2 * C], F32, tag="kvp", name="op")[:, 0:C]
                for cc in range(2):
                    nc.tensor.matmul(
                        op[:], cT_all[:, cc, g * NPO : (g + 1) * NPO],
                        w_sb["wo_s"][:, cc, :],
                        start=(cc == 0), stop=False,
                    )
                bias_mm(op[:], "bo_e", C, NPO)
                o_sb = sb.tile([NPO, C], BF16, tag="o_sb")
                nc.scalar.copy(o_sb[:], op[:])
                nc.gpsimd.dma_start(out_d.ap()[g * NPO : (g + 1) * NPO, :], o_sb[:])

    nc.compile()
    return nc


def _prep_weights(inputs):
    """Host-side weight transforms (all small)."""
    scale = 1.0 / math.sqrt(DK)
    f = lambda k: np.asarray(inputs[k], np.float32)
    tWq, tbq = f("t_Wq") * scale, f("t_bq") * scale
    tWk = f("t_Wk")
    tWv, tbv = f("t_Wv"), f("t_bv")
    tWo, tbo = f("t_Wo"), f("t_bo")
    sWq, sbq = f("s_Wq"), f("s_bq")
    sWk = f("s_Wk")
    sWv, sbv = f("s_Wv"), f("s_bv")
    sWo, sbo = f("s_Wo"), f("s_bo")

    hb = tbv @ tWo + tbo                    # constant part of h_t
    Wq_eff = (tWo @ sWq) * scale
    bq_eff = (hb @ sWq + sbq) * scale
    Wk_eff = tWo @ sWk                      # k bias dropped (softmax-invariant)
    Wv_eff = tWo @ sWv
    cv = hb @ sWv + sbv                     # constant part of v1
    bo_eff = sbo + cv @ sWo

    cm = {
        "wkv_t": _bf16(np.concatenate([tWk, tWv], axis=1).reshape(2, 128, 2 * C)),
        "wq_t": _bf16(tWq.reshape(2, 128, C)),
        "wq_e": _bf16(_pad_cols(Wq_eff).reshape(2, 128, KROW)),
        "wkv_e": _bf16(
            np.concatenate([_pad_cols(Wk_eff), Wv_eff], axis=1).reshape(2, 128, ROW)
        ),
        "wo_s": _bf16(
            sWo.reshape(NH, DK, C)[PERM].reshape(2, 128, C)
        ),
        "bq_t": _bf16(tbq.reshape(1, C)),
        "bq_e": _bf16(_pad_cols(bq_eff).reshape(1, KROW)),
        "bo_e": _bf16(bo_eff.reshape(1, C)),
        "ident": _bf16(np.eye(128, dtype=np.float32)),
        "ones1": _bf16(np.ones((1, 128), np.float32)),
    }
    return cm


def _prep_geometry():
    """Per-core gather indices (local coords) and window masks."""
    masks = np.zeros((CORES, NW, NS * NQ), np.float32)
    for R in range(CR):
        for C4 in range(CC_):
            core = R * CC_ + C4
            for s in range(NS):
                r0, c0 = (s // NS_C) * SQR, (s % NS_C) * SQC
                gr0, gc0 = BR * R + r0, BC * C4 + c0
                wr = gr0 - 2 + np.arange(WR)          # global window rows
                wc = gc0 - 2 + np.arange(WC)
                valid = (wr[:, None] >= 0) & (wr[:, None] < GRID) & \
                        (wc[None, :] >= 0) & (wc[None, :] < GRID)
                qr = gr0 + np.arange(SQR)
                qc = gc0 + np.arange(SQC)
                qrc = np.clip(qr, 2, GRID - 3)
                qcc = np.clip(qc, 2, GRID - 3)
                mrow = (np.abs(wr[:, None] - qrc[None, :]) <= 2)
                mcol = (np.abs(wc[:, None] - qcc[None, :]) <= 2)
                m = (mrow[:, None, :, None] & mcol[None, :, None, :] &
                     valid[:, :, None, None])
                masks[core, :, s * NQ : (s + 1) * NQ] = m.reshape(NW, NQ)
    return _bf16(masks)


def _prep_x(x):
    """x [10000, 5, 256] f32 -> per-core halo-extended channel-major bf16
    chunks [8, G, 128, 2*T*128]."""
    xb = np.asarray(x, np.float32).astype(ml_dtypes.float8_e4m3).reshape(
        GRID, GRID, T, C
    )
    xp = np.zeros((GRID + 4, GRID + 4, T, C), dtype=xb.dtype)
    xp[2 : 2 + GRID, 2 : 2 + GRID] = xb
    out = np.zeros((CORES, G, 128, 2 * T * NP), dtype=xb.dtype)
    for R in range(CR):
        for C4 in range(CC_):
            core = R * CC_ + C4
            blk = xp[BR * R : BR * R + HR, BC * C4 : BC * C4 + HC]  # [54,29,T,C]
            flat = blk.reshape(NH_PIX, T, C)
            flat = np.concatenate(
                [flat, np.zeros((NPAD - NH_PIX, T, C), dtype=xb.dtype)], axis=0
            )
            v = flat.reshape(G, NP, T, 2, 128)
            v = v.transpose(0, 4, 3, 2, 1)      # (g, ch, cc, t, px)
            out[core] = v.reshape(G, 128, 2 * T * NP)
    return out


def _unprep_out(res_list):
    """[8][1250, 256] bf16 -> [10000, 1, 256] f32 global row-major."""
    o = np.stack([np.asarray(r) for r in res_list], axis=0).astype(np.float32)
    v = o.reshape(CR, CC_, BR, BC, C)
    v = v.transpose(0, 2, 1, 3, 4)
    return np.ascontiguousarray(v.reshape(N_FULL, 1, C))


def _make_in_maps(inputs):
    cm = _prep_weights(inputs)
    if "geom" not in _CACHE:
        _CACHE["geom"] = _prep_geometry()
    masks = _CACHE["geom"]
    X = _prep_x(inputs["x"])
    in_maps = []
    for c in range(CORES):
        m = dict(cm)
        m["x"] = X[c]
        m["masks"] = masks[c]
        in_maps.append(m)
    return in_maps


def _get_runner(nc):
    """Build (once) and cache a jitted shard_map callable for the NEFF.

    run_bass_kernel_spmd re-traces and re-jits on every call; caching the
    callable drops warm-call dispatch to the PJRT execute + transfers.
    """
    if "runner" in _CACHE:
        return _CACHE["runner"]
    import jax
    import numpy as jnp_np  # noqa
    from jax.sharding import Mesh, PartitionSpec
    from jax.experimental.shard_map import shard_map
    import concourse.mybir as mb
    from concourse import bass2jax

    bass2jax.install_neuronx_cc_hook()

    in_names, out_names, out_avals, zero_shapes = [], [], [], []
    partition_name = (
        nc.partition_id_tensor.name if nc.partition_id_tensor else None
    )
    for alloc in nc.m.functions[0].allocations:
        if not isinstance(alloc, mb.MemoryLocationSet):
            continue
        name = alloc.memorylocations[0].name
        if alloc.kind == "ExternalInput":
            if name != partition_name:
                in_names.append(name)
        elif alloc.kind == "ExternalOutput":
            shape = tuple(alloc.tensor_shape)
            dtype = mb.dt.np(alloc.dtype)
            out_names.append(name)
            out_avals.append(jax.core.ShapedArray(shape, dtype))
            zero_shapes.append((shape, dtype))
    n_params = len(in_names)
    all_names = list(in_names) + list(out_names)
    if partition_name is not None:
        all_names.append(partition_name)
    donate = tuple(range(n_params, n_params + len(out_names)))

    def _body(*args):
        operands = list(args)
        if partition_name is not None:
            operands.append(bass2jax.partition_id_tensor())
        outs = bass2jax._bass_exec_p.bind(
            *operands,
            out_avals=tuple(out_avals),
            in_names=tuple(all_names),
            out_names=tuple(out_names),
            lowering_input_output_aliases=(),
            sim_require_finite=True,
            sim_require_nnan=True,
            nc=nc,
        )
        return tuple(outs)

    devices = jax.devices()[:CORES]
    mesh = Mesh(np.asarray(devices), ("core",))
    in_specs = (PartitionSpec("core"),) * (n_params + len(out_names))
    out_specs = (PartitionSpec("core"),) * len(out_names)
    sharded = jax.jit(
        shard_map(_body, mesh=mesh, in_specs=in_specs, out_specs=out_specs,
                  check_rep=False),
        donate_argnums=donate, keep_unused=True,
    )

    zfns = [
        jax.jit(
            lambda s=s, dt=dt: jax.numpy.zeros((CORES * s[0], *s[1:]), dt),
            out_shardings=jax.sharding.NamedSharding(mesh, PartitionSpec("core")),
        )
        for s, dt in zero_shapes
    ]
    in_shard = jax.sharding.NamedSharding(mesh, PartitionSpec("core"))

    def run(concat_in):
        args = []
        for n in in_names:
            v = concat_in[n]
            if isinstance(v, tuple):      # (digest, np array): device-cacheable
                key = ("dev", n, v[0])
                if key not in _CACHE:
                    _CACHE[key] = jax.device_put(v[1], in_shard)
                args.append(_CACHE[key])
            else:
                args.append(v)
        zeros = [zf() for zf in zfns]
        outs = sharded(*args, *zeros)
        return {n: outs[i] for i, n in enumerate(out_names)}

    _CACHE["runner"] = run
    return run


def _weights_digest(inputs):
    import hashlib
    h = hashlib.blake2b(digest_size=16)
    for k in sorted(inputs):
        if k not in ("x",):
            h.update(np.ascontiguousarray(inputs[k]).tobytes())
    return h.hexdigest()


def _make_concat_inputs(inputs):
    """Concatenated-along-core-axis input arrays for the cached runner.
    Weight/mask entries are (digest, array) tuples so the runner can keep
    them device-resident across calls."""
    dig = _weights_digest(inputs)
    cm = _prep_weights(inputs)
    if "geom" not in _CACHE:
        _CACHE["geom"] = _prep_geometry()
    masks = _CACHE["geom"]
    X = _prep_x(inputs["x"])
    cat = {}
    for k, v in cm.items():
        full = np.broadcast_to(v, (CORES,) + v.shape).reshape(
            (CORES * v.shape[0],) + v.shape[1:]
        )
        cat[k] = (dig, full)
    cat["x"] = X.reshape(CORES * G, 128, 2 * T * NP)
    cat["masks"] = ("geom", masks.reshape(CORES * NW, NS * NQ))
    return cat


def kernel(**inputs):
    if "nc" not in _CACHE:
        _CACHE["nc"] = _build_graph()
    nc = _CACHE["nc"]
    run = _get_runner(nc)
    cat = _make_concat_inputs(inputs)
    import time as _time
    t0 = _time.perf_counter()
    outs = run(cat)
    out_np = np.asarray(outs["out"])
    _CACHE["last_device_ns"] = (_time.perf_counter() - t0) * 1e9
    o = out_np.reshape(CORES, NLOC, C).astype(np.float32)
    v = o.reshape(CR, CC_, BR, BC, C).transpose(0, 2, 1, 3, 4)
    return np.ascontiguousarray(v.reshape(N_FULL, 1, C))


# revision 34
# speedup vs baseline: 1.0225x; 1.0026x over previous
"""Trainium2 distributed kernel for nn_AttentionFusion (BEV temporal+spatial attention).

Full computation on device across 8 NeuronCores, zero cross-core communication.

Sharding: 2x4 grid of core blocks (50x25 grid pixels per core). Each core
processes its block PLUS a 2-pixel halo (54x29 local region, out-of-grid
pixels zero) so the spatial neighbor windows are always core-local
(redundant temporal compute on the halo instead of a collective).

Per core:
  phase 1 (13 chunks of 128 px): temporal attention. x arrives channel-major
    (host pre-transposed bf16). h_t is never materialized: tWo is composed
    into the spatial projections on the host; k/v biases cancel or fold
    (softmax shift invariance + sum(p)=1). Writes a local kv table
    [1664, 768] = [k padded to 64/head | v] to DRAM.
  phase 2 (25 chunks of 10x5 queries): spatial window attention. The 126-px
    (14x9) window k is fetched channel-major with one transpose-mode
    dma_gather, v pixel-major with a second gather; scores/ctx are dense
    per-head matmuls, masked by a host-precomputed band mask. exp without
    max-subtraction (scores are tiny); softmax denominator via a ones-column
    matmul fused into the ctx pass.
  phase 3 (10 chunks): output projection.

Self-contained: only needs the container toolchain at /opt/trn_rl_repo.
"""

import math
import os
import sys

import numpy as np

sys.path.insert(0, "/opt/trn_rl_repo")

import ml_dtypes  # noqa: E402

import concourse.bass as bass  # noqa: E402
import concourse.bacc as bacc  # noqa: E402
import concourse.mybir as mybir  # noqa: E402
import concourse.tile as tile  # noqa: E402

F32 = mybir.dt.float32
FP8 = mybir.dt.float8e4
BF16 = mybir.dt.bfloat16
I16 = mybir.dt.int16
AX = mybir.AxisListType
ALU = mybir.AluOpType
ACTF = mybir.ActivationFunctionType

# Problem constants
N_FULL = 10000
GRID = 100
T = 5
C = 256
NH = 8
DK = 32
CORES = 8
CR, CC_ = 2, 4             # core grid 2 x 4
BR, BC = 50, 25            # block rows/cols per core
NLOC = BR * BC             # 1250 real pixels per core
HR, HC = BR + 4, BC + 4    # 54 x 29 local region (with halo)
NH_PIX = HR * HC           # 1566
NP = 128
G = (NH_PIX + NP - 1) // NP        # 13 projection chunks
NPAD = G * NP                      # 1664
GO = 10                            # output-projection chunks
NPO = NLOC // GO                   # 125
# spatial chunks: 10 rows x 5 cols of queries
SQR, SQC = 10, 5
NS_R, NS_C = BR // SQR, BC // SQC  # 5 x 5 = 25
NS = NS_R * NS_C
NQ = SQR * SQC             # 50
WR, WC = SQR + 4, SQC + 4  # 14 x 9
NW = WR * WC               # 126
KROW = 384                 # k section: 3 heads per 128-col group at offsets {0,32,64}
# head slot hh (in kq32 tables) -> standard head: hh = 3*(h%3) + h//3
PERM = [0, 3, 6, 1, 4, 7, 2, 5]
VROW = 256
ROW = KROW + VROW          # 768
SEGR = 34                  # rows per overlap segment (2 segments: rows [0,34), [20,54))
SEGP = SEGR * HC           # 986 pixels per segment

_CACHE = {}


def _bf16(a):
    return np.asarray(a, dtype=ml_dtypes.bfloat16)


def _pad_cols(w):
    """[C, 256] -> [C, 384]: head h (32 cols) at 128*(h//3) + 32*(h%3)."""
    w = w.reshape(-1, NH * DK)
    out = np.zeros((w.shape[0], KROW), np.float32)
    for h in range(NH):
        base = 128 * (h // 3) + 32 * (h % 3)
        out[:, base : base + DK] = w[:, DK * h : DK * (h + 1)]
    return out


def _build_graph():
    nc = bacc.Bacc(
        "TRN2",
        target_bir_lowering=False,
        debug=False,
        enable_asserts=False,
        num_devices=CORES,
    )

    # ---------------- I/O ----------------
    x_d = nc.dram_tensor("x", [G, 128, 2 * T * NP], FP8, kind="ExternalInput")
    w_d = {
        "wkv_t": nc.dram_tensor("wkv_t", [2, 128, 2 * C], BF16, kind="ExternalInput"),
        "wq_t": nc.dram_tensor("wq_t", [2, 128, C], BF16, kind="ExternalInput"),
        # spatial q projection, transposed output layout, head-padded [2,128,512]
        "wq_e": nc.dram_tensor("wq_e", [2, 128, KROW], BF16, kind="ExternalInput"),
        # fused (k_padded | v) projection [2, 128, 768]
        "wkv_e": nc.dram_tensor("wkv_e", [2, 128, ROW], BF16, kind="ExternalInput"),
        "wo_s": nc.dram_tensor("wo_s", [2, 128, C], BF16, kind="ExternalInput"),
    }
    b_d = {
        "bq_t": nc.dram_tensor("bq_t", [1, C], BF16, kind="ExternalInput"),
        "bq_e": nc.dram_tensor("bq_e", [1, KROW], BF16, kind="ExternalInput"),
        "bo_e": nc.dram_tensor("bo_e", [1, C], BF16, kind="ExternalInput"),
    }
    ident_d = nc.dram_tensor("ident", [128, 128], BF16, kind="ExternalInput")
    ones1_d = nc.dram_tensor("ones1", [1, 128], BF16, kind="ExternalInput")
    masks_d = nc.dram_tensor("masks", [NW, NS * NQ], BF16, kind="ExternalInput")
    out_d = nc.dram_tensor("out", [NLOC, C], BF16, kind="ExternalOutput")

    with tile.TileContext(nc) as tc:
        with (
            tc.tile_pool(name="const", bufs=1) as cpool,
            tc.tile_pool(name="dram", bufs=1, space="DRAM") as dpool,
            tc.tile_pool(name="sb", bufs=4) as sb,
            tc.tile_pool(name="pkv", bufs=2, space="PSUM") as pkv,
            tc.tile_pool(name="pproj", bufs=2, space="PSUM") as pproj,
            tc.tile_pool(name="ptp", bufs=1, space="PSUM") as ptp,
            tc.tile_pool(name="psc", bufs=2, space="PSUM") as psc,
            tc.tile_pool(name="pcx", bufs=1, space="PSUM") as pcx,
        ):
            v_dram = [
                dpool.tile([SEGP, VROW], BF16, tag=f"v_dram{i}", name=f"v_dram{i}")
                for i in range(2)
            ]

            # ---------- constants ----------
            w_sb = {}
            for n, d in w_d.items():
                t_ = cpool.tile([128, 2, d.shape[2]], BF16, tag=f"w_{n}")
                nc.sync.dma_start(t_[:], d.ap().rearrange("a p c -> p a c"))
                w_sb[n] = t_
            b_sb = {}
            for n, d in b_d.items():
                t_ = cpool.tile([1, d.shape[1]], BF16, tag=f"b_{n}")
                nc.sync.dma_start(t_[:], d.ap())
                b_sb[n] = t_
            ident = cpool.tile([128, 128], BF16, tag="ident")
            nc.sync.dma_start(ident[:], ident_d.ap())
            ones1 = cpool.tile([1, 128], BF16, tag="ones1")
            nc.sync.dma_start(ones1[:], ones1_d.ap())
            onesw = cpool.tile([128, 1], BF16, tag="onesw")
            nc.vector.memset(onesw[:], 1.0)
            masks = cpool.tile([128, NS, NQ], BF16, tag="masks")
            nc.sync.dma_start(
                masks[0:NW, :, :], masks_d.ap().rearrange("w (s q) -> w s q", s=NS)
            )
            kq32 = [
                cpool.tile([32, NH, 2, SEGP], BF16, tag=f"kq32_{i}", name=f"kq32_{i}")
                for i in range(2)
            ]
            cT_all = cpool.tile([128, 2, NLOC], BF16, tag="cT_all")

            def bias_mm(psum_t, b_key, n_out, rows):
                nc.tensor.matmul(
                    psum_t,
                    ones1[0:1, 0:rows],
                    b_sb[b_key][0:1, 0:n_out],
                    start=False,
                    stop=True,
                )

            # ================= PHASE 1: temporal =================
            for g in range(G):
                xg = sb.tile([128, 2, T, NP], BF16, tag="xg")
                nc.gpsimd.dma_start(
                    xg[:], x_d.ap()[g].rearrange("p (a t n) -> p a t n", a=2, t=T)
                )

                kv_sb = sb.tile([NP, T, 2 * C], BF16, tag="kv_sb")
                for t in range(T):
                    kvp = pkv.tile([NP, 2 * C], F32, tag="kvp")
                    for cc in range(2):
                        nc.tensor.matmul(
                            kvp[:],
                            xg[:, cc, t, :],
                            w_sb["wkv_t"][:, cc, :],
                            start=(cc == 0),
                            stop=(cc == 1),
                        )
                    nc.scalar.copy(kv_sb[:, t, :], kvp[:])
                k_sb = kv_sb[:, :, 0:C]
                v_sb = kv_sb[:, :, C : 2 * C]

                qp = pkv.tile([NP, 2 * C], F32, tag="kvp", name="qp")[:, 0:C]
                for cc in range(2):
                    nc.tensor.matmul(
                        qp[:], xg[:, cc, T - 1, :], w_sb["wq_t"][:, cc, :],
                        start=(cc == 0), stop=False,
                    )
                bias_mm(qp[:], "bq_t", C, NP)
                q_sb = sb.tile([NP, C], BF16, tag="q_sb")
                nc.scalar.copy(q_sb[:], qp[:])

                # scores over t (no max subtraction: |s| < 1)
                prod = sb.tile([NP, T, NH, DK], BF16, tag="prod")
                nc.vector.tensor_mul(
                    prod[:],
                    k_sb.rearrange("p t (h d) -> p t h d", h=NH),
                    q_sb[:].rearrange("p (h d) -> p h d", h=NH)
                    .unsqueeze(1)
                    .broadcast_to((NP, T, NH, DK)),
                )
                s_t = sb.tile([NP, T, NH], BF16, tag="s_t")
                with nc.allow_low_precision(reason="temporal scores bf16"):
                    nc.vector.tensor_reduce(s_t[:], prod[:], axis=AX.X, op=ALU.add)
                es = sb.tile([NP, T, NH], F32, tag="es")
                nc.scalar.activation(es[:], s_t[:], ACTF.Exp)
                tsum = sb.tile([NP, NH], F32, tag="tsum")
                nc.vector.tensor_reduce(
                    tsum[:], es[:].rearrange("p t h -> p h t"), axis=AX.X, op=ALU.add
                )
                rinv = sb.tile([NP, NH], F32, tag="trinv")
                nc.vector.reciprocal(rinv[:], tsum[:])
                p_t = sb.tile([NP, T, NH], BF16, tag="p_t")
                nc.vector.tensor_mul(
                    p_t[:], es[:], rinv[:].unsqueeze(1).broadcast_to((NP, T, NH))
                )

                wv = sb.tile([NP, T, C], BF16, tag="wv")
                nc.vector.tensor_mul(
                    wv[:].rearrange("p t (h d) -> p t h d", h=NH),
                    v_sb.rearrange("p t (h d) -> p t h d", h=NH),
                    p_t[:].unsqueeze(3).broadcast_to((NP, T, NH, DK)),
                )
                c1 = sb.tile([NP, 2, C], BF16, tag="c1")
                nc.vector.tensor_add(c1[:], wv[:, 0:2, :], wv[:, 2:4, :])
                c2 = sb.tile([NP, C], BF16, tag="c2")
                nc.vector.tensor_add(c2[:], c1[:, 0, :], c1[:, 1, :])
                ctx = sb.tile([NP, C], BF16, tag="ctx")
                nc.vector.tensor_add(ctx[:], c2[:], wv[:, 4, :])

                # ctxT (channel-major)
                ctp = ptp.tile([128, 2, NP], BF16, tag="ctp")
                for cc in range(2):
                    nc.tensor.transpose(
                        ctp[:, cc, :], ctx[:, cc * 128 : (cc + 1) * 128],
                        ident[0:NP, 0:NP],
                    )
                ctxT = sb.tile([128, 2, NP], BF16, tag="ctxT")
                nc.scalar.copy(ctxT[:], ctp[:])

                # q'T via transposed projection: psum [128, 4, 128]
                qtp = pproj.tile([128, 3, NP], F32, tag="proj", name="qtp")
                for grp in range(3):
                    for cc in range(2):
                        nc.tensor.matmul(
                            qtp[:, grp, :],
                            w_sb["wq_e"][:, cc, 128 * grp : 128 * (grp + 1)],
                            ctxT[:, cc, :],
                            start=(cc == 0),
                            stop=False,
                        )
                    nc.tensor.matmul(
                        qtp[:, grp, :],
                        b_sb["bq_e"][0:1, 128 * grp : 128 * (grp + 1)],
                        ones1[0:1, 0:NP],
                        start=False,
                        stop=True,
                    )
                kq_tmp = sb.tile([128, 3, 2, NP], BF16, tag="kq_tmp")
                nc.vector.tensor_copy(kq_tmp[:, :, 1, :], qtp[:])

                # k1 transposed projection
                ktp = pproj.tile([128, 3, NP], F32, tag="proj", name="ktp")
                for grp in range(3):
                    for cc in range(2):
                        nc.tensor.matmul(
                            ktp[:, grp, :],
                            w_sb["wkv_e"][:, cc, 128 * grp : 128 * (grp + 1)],
                            ctxT[:, cc, :],
                            start=(cc == 0),
                            stop=(cc == 1),
                        )
                nc.scalar.copy(kq_tmp[:, :, 0, :], ktp[:])
                # head regroup: partitions [32m:32m+32) -> head slots [3m:3m+3),
                # scattered into the overlap segments for phase-1/2 pipelining
                glo, ghi = g * NP, min(NH_PIX, (g + 1) * NP)
                for seg in range(2):
                    slo = seg * 20 * HC
                    shi = slo + SEGP
                    lo, hi = max(glo, slo), min(ghi, shi)
                    if lo >= hi:
                        continue
                    for m in range(3):
                        nc.sync.dma_start(
                            kq32[seg][:, 3 * m : min(3 * m + 3, NH), :,
                                      lo - slo : hi - slo],
                            kq_tmp[32 * m : 32 * m + 32,
                                   0 : (3 if m < 2 else 2), :,
                                   lo - glo : hi - glo],
                        )
                vp = pproj.tile([NP, VROW], F32, tag="proj", name="vp")
                for cc in range(2):
                    nc.tensor.matmul(
                        vp[:], ctxT[:, cc, :], w_sb["wkv_e"][:, cc, KROW:ROW],
                        start=(cc == 0), stop=(cc == 1),
                    )
                v1_sb = sb.tile([NP, VROW], BF16, tag="v1_sb")
                nc.scalar.copy(v1_sb[:], vp[:])
                for seg in range(2):
                    slo = seg * 20 * HC
                    shi = slo + SEGP
                    lo, hi = max(glo, slo), min(ghi, shi)
                    if lo < hi:
                        nc.gpsimd.dma_start(
                            v_dram[seg][lo - slo : hi - slo, :],
                            v1_sb[lo - glo : hi - glo, :],
                        )

            PH = os.environ.get("KERNEL_PHASES", "123")
            # ================= PHASE 2: spatial =================
            for s in (range(NS) if "2" in PH else []):
                r0, c0 = (s // NS_C) * SQR, (s % NS_C) * SQC
                seg = 0 if r0 <= 20 else 1
                rs = r0 - 20 * seg
                vw = sb.tile([NW, VROW], BF16, tag="vw")
                nc.gpsimd.dma_start(
                    vw[:],
                    v_dram[seg][:].rearrange("(r c) x -> r c x", r=SEGR)[
                        rs : rs + WR, c0 : c0 + WC, :
                    ],
                )
                kqv = kq32[seg][:].rearrange("p h t (r c) -> p h t r c", r=SEGR)
                kTw = sb.tile([32, NH, NW], BF16, tag="kTw")
                nc.vector.tensor_copy(
                    kTw[:].rearrange("p h (r c) -> p h r c", r=WR),
                    kqv[:, :, 0, rs : rs + WR, c0 : c0 + WC],
                )
                qw = sb.tile([32, NH, NQ], BF16, tag="qw")
                nc.vector.tensor_copy(
                    qw[:].rearrange("p h (r c) -> p h r c", r=SQR),
                    kqv[:, :, 1, 2 + rs : 2 + rs + SQR, 2 + c0 : 2 + c0 + SQC],
                )

                LVL = int(os.environ.get("KERNEL_P2LVL", "9"))
                if LVL < 1:
                    continue
                sc = psc.tile([NW, NH, NQ], F32, tag="sc")
                for h in range(NH):
                    nc.tensor.matmul(
                        sc[:, h, :],
                        kTw[:, h, :],
                        qw[:, h, :],
                        start=True,
                        stop=True,
                    )
                if LVL < 2:
                    continue
                E = sb.tile([NW, NH, NQ], BF16, tag="E")
                nc.scalar.activation(E[:], sc[:], ACTF.Exp)
                E2 = sb.tile([NW, NH, NQ], BF16, tag="E2")
                nc.gpsimd.tensor_mul(
                    E2[:],
                    E[:],
                    masks[0:NW, s, :].unsqueeze(1).broadcast_to((NW, NH, NQ)),
                )

                if LVL < 3:
                    continue
                cx = pcx.tile([NQ, NH, DK + 1], F32, tag="cx")
                for h in range(NH):
                    nc.tensor.matmul(
                        cx[:, h, 0:DK],
                        E2[:, h, :],
                        vw[0:NW, DK * PERM[h] : DK * PERM[h] + DK],
                        start=True,
                        stop=True,
                    )
                    nc.tensor.matmul(
                        cx[:, h, DK : DK + 1],
                        E2[:, h, :],
                        onesw[0:NW, :],
                        start=True,
                        stop=True,
                    )
                srinv = sb.tile([NQ, NH], F32, tag="srinv")
                nc.vector.reciprocal(srinv[:], cx[:, :, DK])
                ctxn = sb.tile([NQ, C], BF16, tag="ctxn")
                nc.vector.tensor_mul(
                    ctxn[:].rearrange("q (h d) -> q h d", h=NH),
                    cx[:, :, 0:DK],
                    srinv[:].unsqueeze(2).broadcast_to((NQ, NH, DK)),
                )
                if LVL < 4:
                    continue
                ntp = ptp.tile([128, 2, NQ], BF16, tag="ctp")
                for cc in range(2):
                    nc.tensor.transpose(
                        ntp[:, cc, :], ctxn[:, cc * 128 : (cc + 1) * 128],
                        ident[0:NQ, 0:NQ],
                    )
                csel = cT_all[:, :, :].rearrange(
                    "p a (r c) -> p a r c", r=BR
                )[:, :, r0 : r0 + SQR, c0 : c0 + SQC]
                nc.scalar.copy(
                    csel[:],
                    ntp[:].rearrange("p a (r c) -> p a r c", r=SQR),
                )

            # ================= PHASE 3: output proj =================
            if "3" not in PH:
                zz = sb.tile([NPO, C], BF16, tag="o_sb", name="zz")
                nc.vector.memset(zz[:], 0.0)
                nc.gpsimd.dma_start(out_d.ap()[0:NPO, :], zz[:])
            for g in (range(GO) if "3" in PH else []):
                op = pkv.tile([NPO, 2 * C], F32, tag="kvp", name="op")[:, 0:C]
                for cc in range(2):
                    nc.tensor.matmul(
                        op[:], cT_all[:, cc, g * NPO : (g + 1) * NPO],
                        w_sb["wo_s"][:, cc, :],
                        start=(cc == 0), stop=False,
                    )
                bias_mm(op[:], "bo_e", C, NPO)
                o_sb = sb.tile([NPO, C], BF16, tag="o_sb")
                nc.scalar.copy(o_sb[:], op[:])
                nc.gpsimd.dma_start(out_d.ap()[g * NPO : (g + 1) * NPO, :], o_sb[:])

    nc.compile()
    return nc


def _prep_weights(inputs):
    """Host-side weight transforms (all small)."""
    scale = 1.0 / math.sqrt(DK)
    f = lambda k: np.asarray(inputs[k], np.float32)
    tWq, tbq = f("t_Wq") * scale, f("t_bq") * scale
    tWk = f("t_Wk")
    tWv, tbv = f("t_Wv"), f("t_bv")
    tWo, tbo = f("t_Wo"), f("t_bo")
    sWq, sbq = f("s_Wq"), f("s_bq")
    sWk = f("s_Wk")
    sWv, sbv = f("s_Wv"), f("s_bv")
    sWo, sbo = f("s_Wo"), f("s_bo")

    hb = tbv @ tWo + tbo                    # constant part of h_t
    Wq_eff = (tWo @ sWq) * scale
    bq_eff = (hb @ sWq + sbq) * scale
    Wk_eff = tWo @ sWk                      # k bias dropped (softmax-invariant)
    Wv_eff = tWo @ sWv
    cv = hb @ sWv + sbv                     # constant part of v1
    bo_eff = sbo + cv @ sWo

    cm = {
        "wkv_t": _bf16(np.concatenate([tWk, tWv], axis=1).reshape(2, 128, 2 * C)),
        "wq_t": _bf16(tWq.reshape(2, 128, C)),
        "wq_e": _bf16(_pad_cols(Wq_eff).reshape(2, 128, KROW)),
        "wkv_e": _bf16(
            np.concatenate([_pad_cols(Wk_eff), Wv_eff], axis=1).reshape(2, 128, ROW)
        ),
        "wo_s": _bf16(
            sWo.reshape(NH, DK, C)[PERM].reshape(2, 128, C)
        ),
        "bq_t": _bf16(tbq.reshape(1, C)),
        "bq_e": _bf16(_pad_cols(bq_eff).reshape(1, KROW)),
        "bo_e": _bf16(bo_eff.reshape(1, C)),
        "ident": _bf16(np.eye(128, dtype=np.float32)),
        "ones1": _bf16(np.ones((1, 128), np.float32)),
    }
    return cm


def _prep_geometry():
    """Per-core gather indices (local coords) and window masks."""
    masks = np.zeros((CORES, NW, NS * NQ), np.float32)
    for R in range(CR):
        for C4 in range(CC_):
            core = R * CC_ + C4
            for s in range(NS):
                r0, c0 = (s // NS_C) * SQR, (s % NS_C) * SQC
                gr0, gc0 = BR * R + r0, BC * C4 + c0
                wr = gr0 - 2 + np.arange(WR)          # global window rows
                wc = gc0 - 2 + np.arange(WC)
                valid = (wr[:, None] >= 0) & (wr[:, None] < GRID) & \
                        (wc[None, :] >= 0) & (wc[None, :] < GRID)
                qr = gr0 + np.arange(SQR)
                qc = gc0 + np.arange(SQC)
                qrc = np.clip(qr, 2, GRID - 3)
                qcc = np.clip(qc, 2, GRID - 3)
                mrow = (np.abs(wr[:, None] - qrc[None, :]) <= 2)
                mcol = (np.abs(wc[:, None] - qcc[None, :]) <= 2)
                m = (mrow[:, None, :, None] & mcol[None, :, None, :] &
                     valid[:, :, None, None])
                masks[core, :, s * NQ : (s + 1) * NQ] = m.reshape(NW, NQ)
    return _bf16(masks)


def _prep_x(x):
    """x [10000, 5, 256] f32 -> per-core halo-extended channel-major bf16
    chunks [8, G, 128, 2*T*128]."""
    xb = np.asarray(x, np.float32).astype(ml_dtypes.float8_e4m3).reshape(
        GRID, GRID, T, C
    )
    xp = np.zeros((GRID + 4, GRID + 4, T, C), dtype=xb.dtype)
    xp[2 : 2 + GRID, 2 : 2 + GRID] = xb
    out = np.zeros((CORES, G, 128, 2 * T * NP), dtype=xb.dtype)
    for R in range(CR):
        for C4 in range(CC_):
            core = R * CC_ + C4
            blk = xp[BR * R : BR * R + HR, BC * C4 : BC * C4 + HC]  # [54,29,T,C]
            flat = blk.reshape(NH_PIX, T, C)
            flat = np.concatenate(
                [flat, np.zeros((NPAD - NH_PIX, T, C), dtype=xb.dtype)], axis=0
            )
            v = flat.reshape(G, NP, T, 2, 128)
            v = v.transpose(0, 4, 3, 2, 1)      # (g, ch, cc, t, px)
            out[core] = v.reshape(G, 128, 2 * T * NP)
    return out


def _unprep_out(res_list):
    """[8][1250, 256] bf16 -> [10000, 1, 256] f32 global row-major."""
    o = np.stack([np.asarray(r) for r in res_list], axis=0).astype(np.float32)
    v = o.reshape(CR, CC_, BR, BC, C)
    v = v.transpose(0, 2, 1, 3, 4)
    return np.ascontiguousarray(v.reshape(N_FULL, 1, C))


def _make_in_maps(inputs):
    cm = _prep_weights(inputs)
    if "geom" not in _CACHE:
        _CACHE["geom"] = _prep_geometry()
    masks = _CACHE["geom"]
    X = _prep_x(inputs["x"])
    in_maps = []
    for c in range(CORES):
        m = dict(cm)
        m["x"] = X[c]
        m["masks"] = masks[c]
        in_maps.append(m)
    return in_maps


def _get_runner(nc):
    """Build (once) and cache a jitted shard_map callable for the NEFF.

    run_bass_kernel_spmd re-traces and re-jits on every call; caching the
    callable drops warm-call dispatch to the PJRT execute + transfers.
    """
    if "runner" in _CACHE:
        return _CACHE["runner"]
    import jax
    import numpy as jnp_np  # noqa
    from jax.sharding import Mesh, PartitionSpec
    from jax.experimental.shard_map import shard_map
    import concourse.mybir as mb
    from concourse import bass2jax

    bass2jax.install_neuronx_cc_hook()

    in_names, out_names, out_avals, zero_shapes = [], [], [], []
    partition_name = (
        nc.partition_id_tensor.name if nc.partition_id_tensor else None
    )
    for alloc in nc.m.functions[0].allocations:
        if not isinstance(alloc, mb.MemoryLocationSet):
            continue
        name = alloc.memorylocations[0].name
        if alloc.kind == "ExternalInput":
            if name != partition_name:
                in_names.append(name)
        elif alloc.kind == "ExternalOutput":
            shape = tuple(alloc.tensor_shape)
            dtype = mb.dt.np(alloc.dtype)
            out_names.append(name)
            out_avals.append(jax.core.ShapedArray(shape, dtype))
            zero_shapes.append((shape, dtype))
    n_params = len(in_names)
    all_names = list(in_names) + list(out_names)
    if partition_name is not None:
        all_names.append(partition_name)
    donate = tuple(range(n_params, n_params + len(out_names)))

    def _body(*args):
        operands = list(args)
        if partition_name is not None:
            operands.append(bass2jax.partition_id_tensor())
        outs = bass2jax._bass_exec_p.bind(
            *operands,
            out_avals=tuple(out_avals),
            in_names=tuple(all_names),
            out_names=tuple(out_names),
            lowering_input_output_aliases=(),
            sim_require_finite=True,
            sim_require_nnan=True,
            nc=nc,
        )
        return tuple(outs)

    devices = jax.devices()[:CORES]
    mesh = Mesh(np.asarray(devices), ("core",))
    in_specs = (PartitionSpec("core"),) * (n_params + len(out_names))
    out_specs = (PartitionSpec("core"),) * len(out_names)
    sharded = jax.jit(
        shard_map(_body, mesh=mesh, in_specs=in_specs, out_specs=out_specs,
                  check_rep=False),
        donate_argnums=donate, keep_unused=True,
    )

    zfns = [
        jax.jit(
            lambda s=s, dt=dt: jax.numpy.zeros((CORES * s[0], *s[1:]), dt),
            out_shardings=jax.sharding.NamedSharding(mesh, PartitionSpec("core")),
        )
        for s, dt in zero_shapes
    ]
    in_shard = jax.sharding.NamedSharding(mesh, PartitionSpec("core"))

    def run(concat_in):
        args = []
        for n in in_names:
            v = concat_in[n]
            if isinstance(v, tuple):      # (digest, np array): device-cacheable
                key = ("dev", n, v[0])
                if key not in _CACHE:
                    _CACHE[key] = jax.device_put(v[1], in_shard)
                args.append(_CACHE[key])
            else:
                args.append(v)
        zeros = [zf() for zf in zfns]
        outs = sharded(*args, *zeros)
        return {n: outs[i] for i, n in enumerate(out_names)}

    _CACHE["runner"] = run
    return run


def _weights_digest(inputs):
    import hashlib
    h = hashlib.blake2b(digest_size=16)
    for k in sorted(inputs):
        if k not in ("x",):
            h.update(np.ascontiguousarray(inputs[k]).tobytes())
    return h.hexdigest()


def _make_concat_inputs(inputs):
    """Concatenated-along-core-axis input arrays for the cached runner.
    Weight/mask entries are (digest, array) tuples so the runner can keep
    them device-resident across calls."""
    dig = _weights_digest(inputs)
    cm = _prep_weights(inputs)
    if "geom" not in _CACHE:
        _CACHE["geom"] = _prep_geometry()
    masks = _CACHE["geom"]
    X = _prep_x(inputs["x"])
    cat = {}
    for k, v in cm.items():
        full = np.broadcast_to(v, (CORES,) + v.shape).reshape(
            (CORES * v.shape[0],) + v.shape[1:]
        )
        cat[k] = (dig, full)
    cat["x"] = X.reshape(CORES * G, 128, 2 * T * NP)
    cat["masks"] = ("geom", masks.reshape(CORES * NW, NS * NQ))
    return cat


def kernel(**inputs):
    if "nc" not in _CACHE:
        _CACHE["nc"] = _build_graph()
    nc = _CACHE["nc"]
    run = _get_runner(nc)
    cat = _make_concat_inputs(inputs)
    import time as _time
    t0 = _time.perf_counter()
    outs = run(cat)
    out_np = np.asarray(outs["out"])
    _CACHE["last_device_ns"] = (_time.perf_counter() - t0) * 1e9
    o = out_np.reshape(CORES, NLOC, C).astype(np.float32)
    v = o.reshape(CR, CC_, BR, BC, C).transpose(0, 2, 1, 3, 4)
    return np.ascontiguousarray(v.reshape(N_FULL, 1, C))


# revision 35
# speedup vs baseline: 1.0233x; 1.0008x over previous
"""Trainium2 distributed kernel for nn_AttentionFusion (BEV temporal+spatial attention).

Full computation on device across 8 NeuronCores, zero cross-core communication.

Sharding: 2x4 grid of core blocks (50x25 grid pixels per core). Each core
processes its block PLUS a 2-pixel halo (54x29 local region, out-of-grid
pixels zero) so the spatial neighbor windows are always core-local
(redundant temporal compute on the halo instead of a collective).

Per core:
  phase 1 (13 chunks of 128 px): temporal attention. x arrives channel-major
    (host pre-transposed bf16). h_t is never materialized: tWo is composed
    into the spatial projections on the host; k/v biases cancel or fold
    (softmax shift invariance + sum(p)=1). Writes a local kv table
    [1664, 768] = [k padded to 64/head | v] to DRAM.
  phase 2 (25 chunks of 10x5 queries): spatial window attention. The 126-px
    (14x9) window k is fetched channel-major with one transpose-mode
    dma_gather, v pixel-major with a second gather; scores/ctx are dense
    per-head matmuls, masked by a host-precomputed band mask. exp without
    max-subtraction (scores are tiny); softmax denominator via a ones-column
    matmul fused into the ctx pass.
  phase 3 (10 chunks): output projection.

Self-contained: only needs the container toolchain at /opt/trn_rl_repo.
"""

import math
import os
import sys

import numpy as np

sys.path.insert(0, "/opt/trn_rl_repo")

import ml_dtypes  # noqa: E402

import concourse.bass as bass  # noqa: E402
import concourse.bacc as bacc  # noqa: E402
import concourse.mybir as mybir  # noqa: E402
import concourse.tile as tile  # noqa: E402

F32 = mybir.dt.float32
FP8 = mybir.dt.float8e4
BF16 = mybir.dt.bfloat16
I16 = mybir.dt.int16
AX = mybir.AxisListType
ALU = mybir.AluOpType
ACTF = mybir.ActivationFunctionType

# Problem constants
N_FULL = 10000
GRID = 100
T = 5
C = 256
NH = 8
DK = 32
CORES = 8
CR, CC_ = 2, 4             # core grid 2 x 4
BR, BC = 50, 25            # block rows/cols per core
NLOC = BR * BC             # 1250 real pixels per core
HR, HC = BR + 4, BC + 4    # 54 x 29 local region (with halo)
NH_PIX = HR * HC           # 1566
NP = 128
G = (NH_PIX + NP - 1) // NP        # 13 projection chunks
NPAD = G * NP                      # 1664
GO = 10                            # output-projection chunks
NPO = NLOC // GO                   # 125
# spatial chunks: 10 rows x 5 cols of queries
SQR, SQC = 10, 5
NS_R, NS_C = BR // SQR, BC // SQC  # 5 x 5 = 25
NS = NS_R * NS_C
NQ = SQR * SQC             # 50
WR, WC = SQR + 4, SQC + 4  # 14 x 9
NW = WR * WC               # 126
KROW = 384                 # k section: 3 heads per 128-col group at offsets {0,32,64}
# head slot hh (in kq32 tables) -> standard head: hh = 3*(h%3) + h//3
PERM = [0, 3, 6, 1, 4, 7, 2, 5]
VROW = 256
ROW = KROW + VROW          # 768
SEGR = 34                  # rows per overlap segment (2 segments: rows [0,34), [20,54))
SEGP = SEGR * HC           # 986 pixels per segment

_CACHE = {}


def _bf16(a):
    return np.asarray(a, dtype=ml_dtypes.bfloat16)


def _pad_cols(w):
    """[C, 256] -> [C, 384]: head h (32 cols) at 128*(h//3) + 32*(h%3)."""
    w = w.reshape(-1, NH * DK)
    out = np.zeros((w.shape[0], KROW), np.float32)
    for h in range(NH):
        base = 128 * (h // 3) + 32 * (h % 3)
        out[:, base : base + DK] = w[:, DK * h : DK * (h + 1)]
    return out


def _build_graph():
    nc = bacc.Bacc(
        "TRN2",
        target_bir_lowering=False,
        debug=False,
        enable_asserts=False,
        num_devices=CORES,
    )

    # ---------------- I/O ----------------
    x_d = nc.dram_tensor("x", [G, 128, 2 * T * NP], FP8, kind="ExternalInput")
    w_d = {
        "wkv_t": nc.dram_tensor("wkv_t", [2, 128, 2 * C], BF16, kind="ExternalInput"),
        "wq_t": nc.dram_tensor("wq_t", [2, 128, C], BF16, kind="ExternalInput"),
        # spatial q projection, transposed output layout, head-padded [2,128,512]
        "wq_e": nc.dram_tensor("wq_e", [2, 128, KROW], BF16, kind="ExternalInput"),
        # fused (k_padded | v) projection [2, 128, 768]
        "wkv_e": nc.dram_tensor("wkv_e", [2, 128, ROW], BF16, kind="ExternalInput"),
        "wo_s": nc.dram_tensor("wo_s", [2, 128, C], BF16, kind="ExternalInput"),
    }
    b_d = {
        "bq_t": nc.dram_tensor("bq_t", [1, C], BF16, kind="ExternalInput"),
        "bq_e": nc.dram_tensor("bq_e", [1, KROW], BF16, kind="ExternalInput"),
        "bo_e": nc.dram_tensor("bo_e", [1, C], BF16, kind="ExternalInput"),
    }
    ident_d = nc.dram_tensor("ident", [128, 128], BF16, kind="ExternalInput")
    ones1_d = nc.dram_tensor("ones1", [1, 128], BF16, kind="ExternalInput")
    masks_d = nc.dram_tensor("masks", [NW, NS * NQ], BF16, kind="ExternalInput")
    out_d = nc.dram_tensor("out", [NLOC, C], BF16, kind="ExternalOutput")

    with tile.TileContext(nc) as tc:
        with (
            tc.tile_pool(name="const", bufs=1) as cpool,
            tc.tile_pool(name="dram", bufs=1, space="DRAM") as dpool,
            tc.tile_pool(name="sb", bufs=4) as sb,
            tc.tile_pool(name="pkv", bufs=2, space="PSUM") as pkv,
            tc.tile_pool(name="pproj", bufs=1, space="PSUM") as pproj,
            tc.tile_pool(name="ptp", bufs=1, space="PSUM") as ptp,
            tc.tile_pool(name="psc", bufs=1, space="PSUM") as psc,
            tc.tile_pool(name="pcx", bufs=1, space="PSUM") as pcx,
        ):
            v_dram = [
                dpool.tile([SEGP, VROW], BF16, tag=f"v_dram{i}", name=f"v_dram{i}")
                for i in range(2)
            ]

            # ---------- constants ----------
            w_sb = {}
            for n, d in w_d.items():
                t_ = cpool.tile([128, 2, d.shape[2]], BF16, tag=f"w_{n}")
                nc.sync.dma_start(t_[:], d.ap().rearrange("a p c -> p a c"))
                w_sb[n] = t_
            b_sb = {}
            for n, d in b_d.items():
                t_ = cpool.tile([1, d.shape[1]], BF16, tag=f"b_{n}")
                nc.sync.dma_start(t_[:], d.ap())
                b_sb[n] = t_
            ident = cpool.tile([128, 128], BF16, tag="ident")
            nc.sync.dma_start(ident[:], ident_d.ap())
            ones1 = cpool.tile([1, 128], BF16, tag="ones1")
            nc.sync.dma_start(ones1[:], ones1_d.ap())
            onesw = cpool.tile([128, 1], BF16, tag="onesw")
            nc.vector.memset(onesw[:], 1.0)
            masks = cpool.tile([128, NS, NQ], BF16, tag="masks")
            nc.sync.dma_start(
                masks[0:NW, :, :], masks_d.ap().rearrange("w (s q) -> w s q", s=NS)
            )
            kq32 = [
                cpool.tile([32, NH, 2, SEGP], BF16, tag=f"kq32_{i}", name=f"kq32_{i}")
                for i in range(2)
            ]
            cT_all = cpool.tile([128, 2, NLOC], BF16, tag="cT_all")

            def bias_mm(psum_t, b_key, n_out, rows):
                nc.tensor.matmul(
                    psum_t,
                    ones1[0:1, 0:rows],
                    b_sb[b_key][0:1, 0:n_out],
                    start=False,
                    stop=True,
                )

            # ================= PHASE 1: temporal =================
            for g in range(G):
                xg = sb.tile([128, 2, T, NP], BF16, tag="xg")
                nc.gpsimd.dma_start(
                    xg[:], x_d.ap()[g].rearrange("p (a t n) -> p a t n", a=2, t=T)
                )

                kv_sb = sb.tile([NP, T, 2 * C], BF16, tag="kv_sb")
                for t0 in range(0, T, 2):
                    nt = min(2, T - t0)
                    kvp = pkv.tile([NP, 2, 2 * C], F32, tag="kvp")
                    for dt_ in range(nt):
                        for cc in range(2):
                            nc.tensor.matmul(
                                kvp[:, dt_, :],
                                xg[:, cc, t0 + dt_, :],
                                w_sb["wkv_t"][:, cc, :],
                                start=(cc == 0),
                                stop=(cc == 1),
                            )
                    nc.scalar.copy(
                        kv_sb[:, t0 : t0 + nt, :], kvp[:, 0:nt, :]
                    )
                k_sb = kv_sb[:, :, 0:C]
                v_sb = kv_sb[:, :, C : 2 * C]

                qp = pkv.tile([NP, 2, 2 * C], F32, tag="kvp", name="qp")[:, 0, 0:C]
                for cc in range(2):
                    nc.tensor.matmul(
                        qp[:], xg[:, cc, T - 1, :], w_sb["wq_t"][:, cc, :],
                        start=(cc == 0), stop=False,
                    )
                bias_mm(qp[:], "bq_t", C, NP)
                q_sb = sb.tile([NP, C], BF16, tag="q_sb")
                nc.scalar.copy(q_sb[:], qp[:])

                # scores over t (no max subtraction: |s| < 1)
                prod = sb.tile([NP, T, NH, DK], BF16, tag="prod")
                nc.vector.tensor_mul(
                    prod[:],
                    k_sb.rearrange("p t (h d) -> p t h d", h=NH),
                    q_sb[:].rearrange("p (h d) -> p h d", h=NH)
                    .unsqueeze(1)
                    .broadcast_to((NP, T, NH, DK)),
                )
                s_t = sb.tile([NP, T, NH], BF16, tag="s_t")
                with nc.allow_low_precision(reason="temporal scores bf16"):
                    nc.vector.tensor_reduce(s_t[:], prod[:], axis=AX.X, op=ALU.add)
                es = sb.tile([NP, T, NH], F32, tag="es")
                nc.scalar.activation(es[:], s_t[:], ACTF.Exp)
                tsum = sb.tile([NP, NH], F32, tag="tsum")
                nc.vector.tensor_reduce(
                    tsum[:], es[:].rearrange("p t h -> p h t"), axis=AX.X, op=ALU.add
                )
                rinv = sb.tile([NP, NH], F32, tag="trinv")
                nc.vector.reciprocal(rinv[:], tsum[:])
                p_t = sb.tile([NP, T, NH], BF16, tag="p_t")
                nc.vector.tensor_mul(
                    p_t[:], es[:], rinv[:].unsqueeze(1).broadcast_to((NP, T, NH))
                )

                wv = sb.tile([NP, T, C], BF16, tag="wv")
                nc.vector.tensor_mul(
                    wv[:].rearrange("p t (h d) -> p t h d", h=NH),
                    v_sb.rearrange("p t (h d) -> p t h d", h=NH),
                    p_t[:].unsqueeze(3).broadcast_to((NP, T, NH, DK)),
                )
                c1 = sb.tile([NP, 2, C], BF16, tag="c1")
                nc.vector.tensor_add(c1[:], wv[:, 0:2, :], wv[:, 2:4, :])
                c2 = sb.tile([NP, C], BF16, tag="c2")
                nc.vector.tensor_add(c2[:], c1[:, 0, :], c1[:, 1, :])
                ctx = sb.tile([NP, C], BF16, tag="ctx")
                nc.vector.tensor_add(ctx[:], c2[:], wv[:, 4, :])

                # ctxT (channel-major)
                ctp = ptp.tile([128, 2, NP], BF16, tag="ctp")
                for cc in range(2):
                    nc.tensor.transpose(
                        ctp[:, cc, :], ctx[:, cc * 128 : (cc + 1) * 128],
                        ident[0:NP, 0:NP],
                    )
                ctxT = sb.tile([128, 2, NP], BF16, tag="ctxT")
                nc.scalar.copy(ctxT[:], ctp[:])

                # q'T via transposed projection: psum [128, 4, 128]
                qtp = pproj.tile([128, 3, NP], F32, tag="proj", name="qtp")
                for grp in range(3):
                    for cc in range(2):
                        nc.tensor.matmul(
                            qtp[:, grp, :],
                            w_sb["wq_e"][:, cc, 128 * grp : 128 * (grp + 1)],
                            ctxT[:, cc, :],
                            start=(cc == 0),
                            stop=False,
                        )
                    nc.tensor.matmul(
                        qtp[:, grp, :],
                        b_sb["bq_e"][0:1, 128 * grp : 128 * (grp + 1)],
                        ones1[0:1, 0:NP],
                        start=False,
                        stop=True,
                    )
                kq_tmp = sb.tile([128, 3, 2, NP], BF16, tag="kq_tmp")
                nc.vector.tensor_copy(kq_tmp[:, :, 1, :], qtp[:])

                # k1 transposed projection
                ktp = pproj.tile([128, 3, NP], F32, tag="proj", name="ktp")
                for grp in range(3):
                    for cc in range(2):
                        nc.tensor.matmul(
                            ktp[:, grp, :],
                            w_sb["wkv_e"][:, cc, 128 * grp : 128 * (grp + 1)],
                            ctxT[:, cc, :],
                            start=(cc == 0),
                            stop=(cc == 1),
                        )
                nc.scalar.copy(kq_tmp[:, :, 0, :], ktp[:])
                # head regroup: partitions [32m:32m+32) -> head slots [3m:3m+3),
                # scattered into the overlap segments for phase-1/2 pipelining
                glo, ghi = g * NP, min(NH_PIX, (g + 1) * NP)
                for seg in range(2):
                    slo = seg * 20 * HC
                    shi = slo + SEGP
                    lo, hi = max(glo, slo), min(ghi, shi)
                    if lo >= hi:
                        continue
                    for m in range(3):
                        nc.sync.dma_start(
                            kq32[seg][:, 3 * m : min(3 * m + 3, NH), :,
                                      lo - slo : hi - slo],
                            kq_tmp[32 * m : 32 * m + 32,
                                   0 : (3 if m < 2 else 2), :,
                                   lo - glo : hi - glo],
                        )
                vp = pproj.tile([NP, VROW], F32, tag="proj", name="vp")
                for cc in range(2):
                    nc.tensor.matmul(
                        vp[:], ctxT[:, cc, :], w_sb["wkv_e"][:, cc, KROW:ROW],
                        start=(cc == 0), stop=(cc == 1),
                    )
                v1_sb = sb.tile([NP, VROW], BF16, tag="v1_sb")
                nc.scalar.copy(v1_sb[:], vp[:])
                for seg in range(2):
                    slo = seg * 20 * HC
                    shi = slo + SEGP
                    lo, hi = max(glo, slo), min(ghi, shi)
                    if lo < hi:
                        nc.gpsimd.dma_start(
                            v_dram[seg][lo - slo : hi - slo, :],
                            v1_sb[lo - glo : hi - glo, :],
                        )

            PH = os.environ.get("KERNEL_PHASES", "123")
            # ================= PHASE 2: spatial =================
            for s in (range(NS) if "2" in PH else []):
                r0, c0 = (s // NS_C) * SQR, (s % NS_C) * SQC
                seg = 0 if r0 <= 20 else 1
                rs = r0 - 20 * seg
                vw = sb.tile([NW, VROW], BF16, tag="vw")
                nc.gpsimd.dma_start(
                    vw[:],
                    v_dram[seg][:].rearrange("(r c) x -> r c x", r=SEGR)[
                        rs : rs + WR, c0 : c0 + WC, :
                    ],
                )
                kqv = kq32[seg][:].rearrange("p h t (r c) -> p h t r c", r=SEGR)
                kTw = sb.tile([32, NH, NW], BF16, tag="kTw")
                nc.vector.tensor_copy(
                    kTw[:].rearrange("p h (r c) -> p h r c", r=WR),
                    kqv[:, :, 0, rs : rs + WR, c0 : c0 + WC],
                )
                qw = sb.tile([32, NH, NQ], BF16, tag="qw")
                nc.vector.tensor_copy(
                    qw[:].rearrange("p h (r c) -> p h r c", r=SQR),
                    kqv[:, :, 1, 2 + rs : 2 + rs + SQR, 2 + c0 : 2 + c0 + SQC],
                )

                LVL = int(os.environ.get("KERNEL_P2LVL", "9"))
                if LVL < 1:
                    continue
                sc = psc.tile([NW, NH, NQ], F32, tag="sc")
                for h in range(NH):
                    nc.tensor.matmul(
                        sc[:, h, :],
                        kTw[:, h, :],
                        qw[:, h, :],
                        start=True,
                        stop=True,
                    )
                if LVL < 2:
                    continue
                E = sb.tile([NW, NH, NQ], BF16, tag="E")
                nc.scalar.activation(E[:], sc[:], ACTF.Exp)
                E2 = sb.tile([NW, NH, NQ], BF16, tag="E2")
                nc.gpsimd.tensor_mul(
                    E2[:],
                    E[:],
                    masks[0:NW, s, :].unsqueeze(1).broadcast_to((NW, NH, NQ)),
                )

                if LVL < 3:
                    continue
                cx = pcx.tile([NQ, NH, DK + 1], F32, tag="cx")
                for h in range(NH):
                    nc.tensor.matmul(
                        cx[:, h, 0:DK],
                        E2[:, h, :],
                        vw[0:NW, DK * PERM[h] : DK * PERM[h] + DK],
                        start=True,
                        stop=True,
                    )
                    nc.tensor.matmul(
                        cx[:, h, DK : DK + 1],
                        E2[:, h, :],
                        onesw[0:NW, :],
                        start=True,
                        stop=True,
                    )
                srinv = sb.tile([NQ, NH], F32, tag="srinv")
                nc.vector.reciprocal(srinv[:], cx[:, :, DK])
                ctxn = sb.tile([NQ, C], BF16, tag="ctxn")
                nc.vector.tensor_mul(
                    ctxn[:].rearrange("q (h d) -> q h d", h=NH),
                    cx[:, :, 0:DK],
                    srinv[:].unsqueeze(2).broadcast_to((NQ, NH, DK)),
                )
                if LVL < 4:
                    continue
                ntp = ptp.tile([128, 2, NQ], BF16, tag="ctp")
                for cc in range(2):
                    nc.tensor.transpose(
                        ntp[:, cc, :], ctxn[:, cc * 128 : (cc + 1) * 128],
                        ident[0:NQ, 0:NQ],
                    )
                csel = cT_all[:, :, :].rearrange(
                    "p a (r c) -> p a r c", r=BR
                )[:, :, r0 : r0 + SQR, c0 : c0 + SQC]
                nc.scalar.copy(
                    csel[:],
                    ntp[:].rearrange("p a (r c) -> p a r c", r=SQR),
                )

            # ================= PHASE 3: output proj =================
            if "3" not in PH:
                zz = sb.tile([NPO, C], BF16, tag="o_sb", name="zz")
                nc.vector.memset(zz[:], 0.0)
                nc.gpsimd.dma_start(out_d.ap()[0:NPO, :], zz[:])
            for g in (range(GO) if "3" in PH else []):
                op = pkv.tile([NPO, 2, 2 * C], F32, tag="kvp", name="op")[:, 0, 0:C]
                for cc in range(2):
                    nc.tensor.matmul(
                        op[:], cT_all[:, cc, g * NPO : (g + 1) * NPO],
                        w_sb["wo_s"][:, cc, :],
                        start=(cc == 0), stop=False,
                    )
                bias_mm(op[:], "bo_e", C, NPO)
                o_sb = sb.tile([NPO, C], BF16, tag="o_sb")
                nc.scalar.copy(o_sb[:], op[:])
                nc.gpsimd.dma_start(out_d.ap()[g * NPO : (g + 1) * NPO, :], o_sb[:])

    nc.compile()
    return nc


def _prep_weights(inputs):
    """Host-side weight transforms (all small)."""
    scale = 1.0 / math.sqrt(DK)
    f = lambda k: np.asarray(inputs[k], np.float32)
    tWq, tbq = f("t_Wq") * scale, f("t_bq") * scale
    tWk = f("t_Wk")
    tWv, tbv = f("t_Wv"), f("t_bv")
    tWo, tbo = f("t_Wo"), f("t_bo")
    sWq, sbq = f("s_Wq"), f("s_bq")
    sWk = f("s_Wk")
    sWv, sbv = f("s_Wv"), f("s_bv")
    sWo, sbo = f("s_Wo"), f("s_bo")

    hb = tbv @ tWo + tbo                    # constant part of h_t
    Wq_eff = (tWo @ sWq) * scale
    bq_eff = (hb @ sWq + sbq) * scale
    Wk_eff = tWo @ sWk                      # k bias dropped (softmax-invariant)
    Wv_eff = tWo @ sWv
    cv = hb @ sWv + sbv                     # constant part of v1
    bo_eff = sbo + cv @ sWo

    cm = {
        "wkv_t": _bf16(np.concatenate([tWk, tWv], axis=1).reshape(2, 128, 2 * C)),
        "wq_t": _bf16(tWq.reshape(2, 128, C)),
        "wq_e": _bf16(_pad_cols(Wq_eff).reshape(2, 128, KROW)),
        "wkv_e": _bf16(
            np.concatenate([_pad_cols(Wk_eff), Wv_eff], axis=1).reshape(2, 128, ROW)
        ),
        "wo_s": _bf16(
            sWo.reshape(NH, DK, C)[PERM].reshape(2, 128, C)
        ),
        "bq_t": _bf16(tbq.reshape(1, C)),
        "bq_e": _bf16(_pad_cols(bq_eff).reshape(1, KROW)),
        "bo_e": _bf16(bo_eff.reshape(1, C)),
        "ident": _bf16(np.eye(128, dtype=np.float32)),
        "ones1": _bf16(np.ones((1, 128), np.float32)),
    }
    return cm


def _prep_geometry():
    """Per-core gather indices (local coords) and window masks."""
    masks = np.zeros((CORES, NW, NS * NQ), np.float32)
    for R in range(CR):
        for C4 in range(CC_):
            core = R * CC_ + C4
            for s in range(NS):
                r0, c0 = (s // NS_C) * SQR, (s % NS_C) * SQC
                gr0, gc0 = BR * R + r0, BC * C4 + c0
                wr = gr0 - 2 + np.arange(WR)          # global window rows
                wc = gc0 - 2 + np.arange(WC)
                valid = (wr[:, None] >= 0) & (wr[:, None] < GRID) & \
                        (wc[None, :] >= 0) & (wc[None, :] < GRID)
                qr = gr0 + np.arange(SQR)
                qc = gc0 + np.arange(SQC)
                qrc = np.clip(qr, 2, GRID - 3)
                qcc = np.clip(qc, 2, GRID - 3)
                mrow = (np.abs(wr[:, None] - qrc[None, :]) <= 2)
                mcol = (np.abs(wc[:, None] - qcc[None, :]) <= 2)
                m = (mrow[:, None, :, None] & mcol[None, :, None, :] &
                     valid[:, :, None, None])
                masks[core, :, s * NQ : (s + 1) * NQ] = m.reshape(NW, NQ)
    return _bf16(masks)


def _prep_x(x):
    """x [10000, 5, 256] f32 -> per-core halo-extended channel-major bf16
    chunks [8, G, 128, 2*T*128]."""
    xb = np.asarray(x, np.float32).astype(ml_dtypes.float8_e4m3).reshape(
        GRID, GRID, T, C
    )
    xp = np.zeros((GRID + 4, GRID + 4, T, C), dtype=xb.dtype)
    xp[2 : 2 + GRID, 2 : 2 + GRID] = xb
    out = np.zeros((CORES, G, 128, 2 * T * NP), dtype=xb.dtype)
    for R in range(CR):
        for C4 in range(CC_):
            core = R * CC_ + C4
            blk = xp[BR * R : BR * R + HR, BC * C4 : BC * C4 + HC]  # [54,29,T,C]
            flat = blk.reshape(NH_PIX, T, C)
            flat = np.concatenate(
                [flat, np.zeros((NPAD - NH_PIX, T, C), dtype=xb.dtype)], axis=0
            )
            v = flat.reshape(G, NP, T, 2, 128)
            v = v.transpose(0, 4, 3, 2, 1)      # (g, ch, cc, t, px)
            out[core] = v.reshape(G, 128, 2 * T * NP)
    return out


def _unprep_out(res_list):
    """[8][1250, 256] bf16 -> [10000, 1, 256] f32 global row-major."""
    o = np.stack([np.asarray(r) for r in res_list], axis=0).astype(np.float32)
    v = o.reshape(CR, CC_, BR, BC, C)
    v = v.transpose(0, 2, 1, 3, 4)
    return np.ascontiguousarray(v.reshape(N_FULL, 1, C))


def _make_in_maps(inputs):
    cm = _prep_weights(inputs)
    if "geom" not in _CACHE:
        _CACHE["geom"] = _prep_geometry()
    masks = _CACHE["geom"]
    X = _prep_x(inputs["x"])
    in_maps = []
    for c in range(CORES):
        m = dict(cm)
        m["x"] = X[c]
        m["masks"] = masks[c]
        in_maps.append(m)
    return in_maps


def _get_runner(nc):
    """Build (once) and cache a jitted shard_map callable for the NEFF.

    run_bass_kernel_spmd re-traces and re-jits on every call; caching the
    callable drops warm-call dispatch to the PJRT execute + transfers.
    """
    if "runner" in _CACHE:
        return _CACHE["runner"]
    import jax
    import numpy as jnp_np  # noqa
    from jax.sharding import Mesh, PartitionSpec
    from jax.experimental.shard_map import shard_map
    import concourse.mybir as mb
    from concourse import bass2jax

    bass2jax.install_neuronx_cc_hook()

    in_names, out_names, out_avals, zero_shapes = [], [], [], []
    partition_name = (
        nc.partition_id_tensor.name if nc.partition_id_tensor else None
    )
    for alloc in nc.m.functions[0].allocations:
        if not isinstance(alloc, mb.MemoryLocationSet):
            continue
        name = alloc.memorylocations[0].name
        if alloc.kind == "ExternalInput":
            if name != partition_name:
                in_names.append(name)
        elif alloc.kind == "ExternalOutput":
            shape = tuple(alloc.tensor_shape)
            dtype = mb.dt.np(alloc.dtype)
            out_names.append(name)
            out_avals.append(jax.core.ShapedArray(shape, dtype))
            zero_shapes.append((shape, dtype))
    n_params = len(in_names)
    all_names = list(in_names) + list(out_names)
    if partition_name is not None:
        all_names.append(partition_name)
    donate = tuple(range(n_params, n_params + len(out_names)))

    def _body(*args):
        operands = list(args)
        if partition_name is not None:
            operands.append(bass2jax.partition_id_tensor())
        outs = bass2jax._bass_exec_p.bind(
            *operands,
            out_avals=tuple(out_avals),
            in_names=tuple(all_names),
            out_names=tuple(out_names),
            lowering_input_output_aliases=(),
            sim_require_finite=True,
            sim_require_nnan=True,
            nc=nc,
        )
        return tuple(outs)

    devices = jax.devices()[:CORES]
    mesh = Mesh(np.asarray(devices), ("core",))
    in_specs = (PartitionSpec("core"),) * (n_params + len(out_names))
    out_specs = (PartitionSpec("core"),) * len(out_names)
    sharded = jax.jit(
        shard_map(_body, mesh=mesh, in_specs=in_specs, out_specs=out_specs,
                  check_rep=False),
        donate_argnums=donate, keep_unused=True,
    )

    zfns = [
        jax.jit(
            lambda s=s, dt=dt: jax.numpy.zeros((CORES * s[0], *s[1:]), dt),
            out_shardings=jax.sharding.NamedSharding(mesh, PartitionSpec("core")),
        )
        for s, dt in zero_shapes
    ]
    in_shard = jax.sharding.NamedSharding(mesh, PartitionSpec("core"))

    def run(concat_in):
        args = []
        for n in in_names:
            v = concat_in[n]
            if isinstance(v, tuple):      # (digest, np array): device-cacheable
                key = ("dev", n, v[0])
                if key not in _CACHE:
                    _CACHE[key] = jax.device_put(v[1], in_shard)
                args.append(_CACHE[key])
            else:
                args.append(v)
        zeros = [zf() for zf in zfns]
        outs = sharded(*args, *zeros)
        return {n: outs[i] for i, n in enumerate(out_names)}

    _CACHE["runner"] = run
    return run


def _weights_digest(inputs):
    import hashlib
    h = hashlib.blake2b(digest_size=16)
    for k in sorted(inputs):
        if k not in ("x",):
            h.update(np.ascontiguousarray(inputs[k]).tobytes())
    return h.hexdigest()


def _make_concat_inputs(inputs):
    """Concatenated-along-core-axis input arrays for the cached runner.
    Weight/mask entries are (digest, array) tuples so the runner can keep
    them device-resident across calls."""
    dig = _weights_digest(inputs)
    cm = _prep_weights(inputs)
    if "geom" not in _CACHE:
        _CACHE["geom"] = _prep_geometry()
    masks = _CACHE["geom"]
    X = _prep_x(inputs["x"])
    cat = {}
    for k, v in cm.items():
        full = np.broadcast_to(v, (CORES,) + v.shape).reshape(
            (CORES * v.shape[0],) + v.shape[1:]
        )
        cat[k] = (dig, full)
    cat["x"] = X.reshape(CORES * G, 128, 2 * T * NP)
    cat["masks"] = ("geom", masks.reshape(CORES * NW, NS * NQ))
    return cat


def kernel(**inputs):
    if "nc" not in _CACHE:
        _CACHE["nc"] = _build_graph()
    nc = _CACHE["nc"]
    run = _get_runner(nc)
    cat = _make_concat_inputs(inputs)
    import time as _time
    t0 = _time.perf_counter()
    outs = run(cat)
    out_np = np.asarray(outs["out"])
    _CACHE["last_device_ns"] = (_time.perf_counter() - t0) * 1e9
    o = out_np.reshape(CORES, NLOC, C).astype(np.float32)
    v = o.reshape(CR, CC_, BR, BC, C).transpose(0, 2, 1, 3, 4)
    return np.ascontiguousarray(v.reshape(N_FULL, 1, C))


# revision 38
# speedup vs baseline: 1.0244x; 1.0011x over previous
"""Trainium2 distributed kernel for nn_AttentionFusion (BEV temporal+spatial attention).

Full computation on device across 8 NeuronCores, zero cross-core communication.

Sharding: 2x4 grid of core blocks (50x25 grid pixels per core). Each core
processes its block PLUS a 2-pixel halo (54x29 local region, out-of-grid
pixels zero) so the spatial neighbor windows are always core-local
(redundant temporal compute on the halo instead of a collective).

Per core:
  phase 1 (13 chunks of 128 px): temporal attention. x arrives channel-major
    (host pre-transposed bf16). h_t is never materialized: tWo is composed
    into the spatial projections on the host; k/v biases cancel or fold
    (softmax shift invariance + sum(p)=1). Writes a local kv table
    [1664, 768] = [k padded to 64/head | v] to DRAM.
  phase 2 (25 chunks of 10x5 queries): spatial window attention. The 126-px
    (14x9) window k is fetched channel-major with one transpose-mode
    dma_gather, v pixel-major with a second gather; scores/ctx are dense
    per-head matmuls, masked by a host-precomputed band mask. exp without
    max-subtraction (scores are tiny); softmax denominator via a ones-column
    matmul fused into the ctx pass.
  phase 3 (10 chunks): output projection.

Self-contained: only needs the container toolchain at /opt/trn_rl_repo.
"""

import math
import os
import sys

import numpy as np

sys.path.insert(0, "/opt/trn_rl_repo")

import ml_dtypes  # noqa: E402

import concourse.bass as bass  # noqa: E402
import concourse.bacc as bacc  # noqa: E402
import concourse.mybir as mybir  # noqa: E402
import concourse.tile as tile  # noqa: E402

F32 = mybir.dt.float32
FP8 = mybir.dt.float8e4
BF16 = mybir.dt.bfloat16
I16 = mybir.dt.int16
AX = mybir.AxisListType
ALU = mybir.AluOpType
ACTF = mybir.ActivationFunctionType

# Problem constants
N_FULL = 10000
GRID = 100
T = 5
C = 256
NH = 8
DK = 32
CORES = 8
CR, CC_ = 2, 4             # core grid 2 x 4
BR, BC = 50, 25            # block rows/cols per core
NLOC = BR * BC             # 1250 real pixels per core
HR, HC = BR + 4, BC + 4    # 54 x 29 local region (with halo)
NH_PIX = HR * HC           # 1566
NP = 128
G = (NH_PIX + NP - 1) // NP        # 13 projection chunks
NPAD = G * NP                      # 1664
GO = 10                            # output-projection chunks
NPO = NLOC // GO                   # 125
# spatial chunks: 10 rows x 5 cols of queries
SQR, SQC = 10, 5
NS_R, NS_C = BR // SQR, BC // SQC  # 5 x 5 = 25
NS = NS_R * NS_C
NQ = SQR * SQC             # 50
WR, WC = SQR + 4, SQC + 4  # 14 x 9
NW = WR * WC               # 126
KROW = 384                 # k section: 3 heads per 128-col group at offsets {0,32,64}
# head slot hh (in kq32 tables) -> standard head: hh = 3*(h%3) + h//3
PERM = [0, 3, 6, 1, 4, 7, 2, 5]
VROW = 256
ROW = KROW + VROW          # 768
SEGR = 34                  # rows per overlap segment (2 segments: rows [0,34), [20,54))
SEGP = SEGR * HC           # 986 pixels per segment

_CACHE = {}


def _bf16(a):
    return np.asarray(a, dtype=ml_dtypes.bfloat16)


def _pad_cols(w):
    """[C, 256] -> [C, 384]: head h (32 cols) at 128*(h//3) + 32*(h%3)."""
    w = w.reshape(-1, NH * DK)
    out = np.zeros((w.shape[0], KROW), np.float32)
    for h in range(NH):
        base = 128 * (h // 3) + 32 * (h % 3)
        out[:, base : base + DK] = w[:, DK * h : DK * (h + 1)]
    return out


def _build_graph():
    nc = bacc.Bacc(
        "TRN2",
        target_bir_lowering=False,
        debug=False,
        enable_asserts=False,
        num_devices=CORES,
    )

    # ---------------- I/O ----------------
    x_d = nc.dram_tensor("x", [G, 128, 2 * T * NP], FP8, kind="ExternalInput")
    w_d = {
        "wkv_t": nc.dram_tensor("wkv_t", [2, 128, 2 * C], BF16, kind="ExternalInput"),
        "wq_t": nc.dram_tensor("wq_t", [2, 128, C], BF16, kind="ExternalInput"),
        # spatial q projection, transposed output layout, head-padded [2,128,512]
        "wq_e": nc.dram_tensor("wq_e", [2, 128, KROW], BF16, kind="ExternalInput"),
        # fused (k_padded | v) projection [2, 128, 768]
        "wkv_e": nc.dram_tensor("wkv_e", [2, 128, ROW], BF16, kind="ExternalInput"),
        "wo_s": nc.dram_tensor("wo_s", [2, 128, C], BF16, kind="ExternalInput"),
    }
    b_d = {
        "bq_t": nc.dram_tensor("bq_t", [1, C], BF16, kind="ExternalInput"),
        "bq_e": nc.dram_tensor("bq_e", [1, KROW], BF16, kind="ExternalInput"),
        "bo_e": nc.dram_tensor("bo_e", [1, C], BF16, kind="ExternalInput"),
    }
    ident_d = nc.dram_tensor("ident", [128, 128], BF16, kind="ExternalInput")
    ones1_d = nc.dram_tensor("ones1", [1, 128], BF16, kind="ExternalInput")
    masks_d = nc.dram_tensor("masks", [NW, NS * NQ], BF16, kind="ExternalInput")
    out_d = nc.dram_tensor("out", [NLOC, C], BF16, kind="ExternalOutput")

    with tile.TileContext(nc) as tc:
        with (
            tc.tile_pool(name="const", bufs=1) as cpool,
            tc.tile_pool(name="dram", bufs=1, space="DRAM") as dpool,
            tc.tile_pool(name="sb", bufs=4) as sb,
            tc.tile_pool(name="pkv", bufs=2, space="PSUM") as pkv,
            tc.tile_pool(name="pproj", bufs=1, space="PSUM") as pproj,
            tc.tile_pool(name="ptp", bufs=1, space="PSUM") as ptp,
            tc.tile_pool(name="psc", bufs=1, space="PSUM") as psc,
            tc.tile_pool(name="pcx", bufs=1, space="PSUM") as pcx,
        ):
            v_dram = [
                dpool.tile([SEGP, VROW], BF16, tag=f"v_dram{i}", name=f"v_dram{i}")
                for i in range(2)
            ]

            # ---------- constants ----------
            w_sb = {}
            for n, d in w_d.items():
                t_ = cpool.tile([128, 2, d.shape[2]], BF16, tag=f"w_{n}")
                nc.sync.dma_start(t_[:], d.ap().rearrange("a p c -> p a c"))
                w_sb[n] = t_
            b_sb = {}
            for n, d in b_d.items():
                t_ = cpool.tile([1, d.shape[1]], BF16, tag=f"b_{n}")
                nc.sync.dma_start(t_[:], d.ap())
                b_sb[n] = t_
            ident = cpool.tile([128, 128], BF16, tag="ident")
            nc.sync.dma_start(ident[:], ident_d.ap())
            ones1 = cpool.tile([1, 128], BF16, tag="ones1")
            nc.sync.dma_start(ones1[:], ones1_d.ap())
            onesw = cpool.tile([128, 1], BF16, tag="onesw")
            nc.vector.memset(onesw[:], 1.0)
            masks = cpool.tile([128, NS, NQ], BF16, tag="masks")
            nc.sync.dma_start(
                masks[0:NW, :, :], masks_d.ap().rearrange("w (s q) -> w s q", s=NS)
            )
            kq32 = [
                cpool.tile([32, NH, 2, SEGP], BF16, tag=f"kq32_{i}", name=f"kq32_{i}")
                for i in range(2)
            ]
            cT_all = cpool.tile([128, 2, NLOC], BF16, tag="cT_all")

            def bias_mm(psum_t, b_key, n_out, rows):
                nc.tensor.matmul(
                    psum_t,
                    ones1[0:1, 0:rows],
                    b_sb[b_key][0:1, 0:n_out],
                    start=False,
                    stop=True,
                )

            # ================= PHASE 1: temporal =================
            for g in range(G):
                xg = sb.tile([128, 2, T, NP], BF16, tag="xg")
                nc.gpsimd.dma_start(
                    xg[:], x_d.ap()[g].rearrange("p (a t n) -> p a t n", a=2, t=T)
                )

                kv_sb = sb.tile([NP, T, 2 * C], BF16, tag="kv_sb")
                for t0 in range(0, T, 2):
                    nt = min(2, T - t0)
                    kvp = pkv.tile([NP, 2, 2 * C], F32, tag="kvp")
                    for dt_ in range(nt):
                        for cc in range(2):
                            nc.tensor.matmul(
                                kvp[:, dt_, :],
                                xg[:, cc, t0 + dt_, :],
                                w_sb["wkv_t"][:, cc, :],
                                start=(cc == 0),
                                stop=(cc == 1),
                            )
                    nc.scalar.copy(
                        kv_sb[:, t0 : t0 + nt, :], kvp[:, 0:nt, :]
                    )
                k_sb = kv_sb[:, :, 0:C]
                v_sb = kv_sb[:, :, C : 2 * C]

                qp = pkv.tile([NP, 2, 2 * C], F32, tag="kvp", name="qp")[:, 0, 0:C]
                for cc in range(2):
                    nc.tensor.matmul(
                        qp[:], xg[:, cc, T - 1, :], w_sb["wq_t"][:, cc, :],
                        start=(cc == 0), stop=False,
                    )
                bias_mm(qp[:], "bq_t", C, NP)
                q_sb = sb.tile([NP, C], BF16, tag="q_sb")
                nc.scalar.copy(q_sb[:], qp[:])

                # scores over t (no max subtraction: |s| < 1)
                prod = sb.tile([NP, T, NH, DK], BF16, tag="prod")
                nc.vector.tensor_mul(
                    prod[:],
                    k_sb.rearrange("p t (h d) -> p t h d", h=NH),
                    q_sb[:].rearrange("p (h d) -> p h d", h=NH)
                    .unsqueeze(1)
                    .broadcast_to((NP, T, NH, DK)),
                )
                s_t = sb.tile([NP, T, NH], BF16, tag="s_t")
                with nc.allow_low_precision(reason="temporal scores bf16"):
                    nc.vector.tensor_reduce(s_t[:], prod[:], axis=AX.X, op=ALU.add)
                es = sb.tile([NP, T, NH], F32, tag="es")
                nc.scalar.activation(es[:], s_t[:], ACTF.Exp)
                tsum = sb.tile([NP, NH], F32, tag="tsum")
                nc.vector.tensor_reduce(
                    tsum[:], es[:].rearrange("p t h -> p h t"), axis=AX.X, op=ALU.add
                )
                rinv = sb.tile([NP, NH], F32, tag="trinv")
                nc.vector.reciprocal(rinv[:], tsum[:])
                p_t = sb.tile([NP, T, NH], BF16, tag="p_t")
                nc.vector.tensor_mul(
                    p_t[:], es[:], rinv[:].unsqueeze(1).broadcast_to((NP, T, NH))
                )

                wv = sb.tile([NP, T, C], BF16, tag="wv")
                nc.vector.tensor_mul(
                    wv[:].rearrange("p t (h d) -> p t h d", h=NH),
                    v_sb.rearrange("p t (h d) -> p t h d", h=NH),
                    p_t[:].unsqueeze(3).broadcast_to((NP, T, NH, DK)),
                )
                c1 = sb.tile([NP, 2, C], BF16, tag="c1")
                nc.vector.tensor_add(c1[:], wv[:, 0:2, :], wv[:, 2:4, :])
                c2 = sb.tile([NP, C], BF16, tag="c2")
                nc.vector.tensor_add(c2[:], c1[:, 0, :], c1[:, 1, :])
                ctx = sb.tile([NP, C], BF16, tag="ctx")
                nc.vector.tensor_add(ctx[:], c2[:], wv[:, 4, :])

                # ctxT (channel-major)
                ctp = ptp.tile([128, 2, NP], BF16, tag="ctp")
                for cc in range(2):
                    nc.tensor.transpose(
                        ctp[:, cc, :], ctx[:, cc * 128 : (cc + 1) * 128],
                        ident[0:NP, 0:NP],
                    )
                ctxT = sb.tile([128, 2, NP], BF16, tag="ctxT")
                nc.scalar.copy(ctxT[:], ctp[:])

                # q'T via transposed projection (skipped for the last chunk:
                # rows >= 53 are halo/pad, their q' is never read)
                need_q = g * NP < (HR - 2) * HC - 2 * HC + 0 or True
                need_q = g * NP < 1535
                qtp = pproj.tile([128, 3, NP], F32, tag="proj", name="qtp")
                for grp in (range(3) if need_q else []):
                    for cc in range(2):
                        nc.tensor.matmul(
                            qtp[:, grp, :],
                            w_sb["wq_e"][:, cc, 128 * grp : 128 * (grp + 1)],
                            ctxT[:, cc, :],
                            start=(cc == 0),
                            stop=False,
                        )
                    nc.tensor.matmul(
                        qtp[:, grp, :],
                        b_sb["bq_e"][0:1, 128 * grp : 128 * (grp + 1)],
                        ones1[0:1, 0:NP],
                        start=False,
                        stop=True,
                    )
                kq_tmp = sb.tile([128, 3, 2, NP], BF16, tag="kq_tmp")
                if need_q:
                    nc.vector.tensor_copy(kq_tmp[:, :, 1, :], qtp[:])

                # k1 transposed projection
                ktp = pproj.tile([128, 3, NP], F32, tag="proj", name="ktp")
                for grp in range(3):
                    for cc in range(2):
                        nc.tensor.matmul(
                            ktp[:, grp, :],
                            w_sb["wkv_e"][:, cc, 128 * grp : 128 * (grp + 1)],
                            ctxT[:, cc, :],
                            start=(cc == 0),
                            stop=(cc == 1),
                        )
                nc.scalar.copy(kq_tmp[:, :, 0, :], ktp[:])
                # head regroup: partitions [32m:32m+32) -> head slots [3m:3m+3),
                # scattered into the overlap segments for phase-1/2 pipelining
                glo, ghi = g * NP, min(NH_PIX, (g + 1) * NP)
                for seg in range(2):
                    slo = seg * 20 * HC
                    shi = slo + SEGP
                    lo, hi = max(glo, slo), min(ghi, shi)
                    if lo >= hi:
                        continue
                    nkq = 2 if need_q else 1
                    for m in range(3):
                        nc.sync.dma_start(
                            kq32[seg][:, 3 * m : min(3 * m + 3, NH), 0:nkq,
                                      lo - slo : hi - slo],
                            kq_tmp[32 * m : 32 * m + 32,
                                   0 : (3 if m < 2 else 2), 0:nkq,
                                   lo - glo : hi - glo],
                        )
                vp = pproj.tile([NP, VROW], F32, tag="proj", name="vp")
                for cc in range(2):
                    nc.tensor.matmul(
                        vp[:], ctxT[:, cc, :], w_sb["wkv_e"][:, cc, KROW:ROW],
                        start=(cc == 0), stop=(cc == 1),
                    )
                v1_sb = sb.tile([NP, VROW], BF16, tag="v1_sb")
                nc.scalar.copy(v1_sb[:], vp[:])
                for seg in range(2):
                    slo = seg * 20 * HC
                    shi = slo + SEGP
                    lo, hi = max(glo, slo), min(ghi, shi)
                    if lo < hi:
                        nc.gpsimd.dma_start(
                            v_dram[seg][lo - slo : hi - slo, :],
                            v1_sb[lo - glo : hi - glo, :],
                        )

            PH = os.environ.get("KERNEL_PHASES", "123")
            # ================= PHASE 2: spatial =================
            for s in (range(NS) if "2" in PH else []):
                r0, c0 = (s // NS_C) * SQR, (s % NS_C) * SQC
                seg = 0 if r0 <= 20 else 1
                rs = r0 - 20 * seg
                vw = sb.tile([NW, VROW], BF16, tag="vw")
                nc.gpsimd.dma_start(
                    vw[:],
                    v_dram[seg][:].rearrange("(r c) x -> r c x", r=SEGR)[
                        rs : rs + WR, c0 : c0 + WC, :
                    ],
                )
                kqv = kq32[seg][:].rearrange("p h t (r c) -> p h t r c", r=SEGR)
                kTw = sb.tile([32, NH, NW], BF16, tag="kTw")
                nc.vector.tensor_copy(
                    kTw[:].rearrange("p h (r c) -> p h r c", r=WR),
                    kqv[:, :, 0, rs : rs + WR, c0 : c0 + WC],
                )
                qw = sb.tile([32, NH, NQ], BF16, tag="qw")
                nc.vector.tensor_copy(
                    qw[:].rearrange("p h (r c) -> p h r c", r=SQR),
                    kqv[:, :, 1, 2 + rs : 2 + rs + SQR, 2 + c0 : 2 + c0 + SQC],
                )

                LVL = int(os.environ.get("KERNEL_P2LVL", "9"))
                if LVL < 1:
                    continue
                sc = psc.tile([NW, NH, NQ], F32, tag="sc")
                for h in range(NH):
                    nc.tensor.matmul(
                        sc[:, h, :],
                        kTw[:, h, :],
                        qw[:, h, :],
                        start=True,
                        stop=True,
                    )
                if LVL < 2:
                    continue
                E = sb.tile([NW, NH, NQ], BF16, tag="E")
                nc.scalar.activation(E[:], sc[:], ACTF.Exp)
                E2 = sb.tile([NW, NH, NQ], BF16, tag="E2")
                nc.gpsimd.tensor_mul(
                    E2[:],
                    E[:],
                    masks[0:NW, s, :].unsqueeze(1).broadcast_to((NW, NH, NQ)),
                )

                if LVL < 3:
                    continue
                cx = pcx.tile([NQ, NH, DK + 1], F32, tag="cx")
                for h in range(NH):
                    nc.tensor.matmul(
                        cx[:, h, 0:DK],
                        E2[:, h, :],
                        vw[0:NW, DK * PERM[h] : DK * PERM[h] + DK],
                        start=True,
                        stop=True,
                    )
                    nc.tensor.matmul(
                        cx[:, h, DK : DK + 1],
                        E2[:, h, :],
                        onesw[0:NW, :],
                        start=True,
                        stop=True,
                    )
                srinv = sb.tile([NQ, NH], F32, tag="srinv")
                nc.vector.reciprocal(srinv[:], cx[:, :, DK])
                ctxn = sb.tile([NQ, C], BF16, tag="ctxn")
                nc.vector.tensor_mul(
                    ctxn[:].rearrange("q (h d) -> q h d", h=NH),
                    cx[:, :, 0:DK],
                    srinv[:].unsqueeze(2).broadcast_to((NQ, NH, DK)),
                )
                if LVL < 4:
                    continue
                ntp = ptp.tile([128, 2, NQ], BF16, tag="ctp")
                for cc in range(2):
                    nc.tensor.transpose(
                        ntp[:, cc, :], ctxn[:, cc * 128 : (cc + 1) * 128],
                        ident[0:NQ, 0:NQ],
                    )
                csel = cT_all[:, :, :].rearrange(
                    "p a (r c) -> p a r c", r=BR
                )[:, :, r0 : r0 + SQR, c0 : c0 + SQC]
                nc.scalar.copy(
                    csel[:],
                    ntp[:].rearrange("p a (r c) -> p a r c", r=SQR),
                )

            # ================= PHASE 3: output proj =================
            if "3" not in PH:
                zz = sb.tile([NPO, C], BF16, tag="o_sb", name="zz")
                nc.vector.memset(zz[:], 0.0)
                nc.gpsimd.dma_start(out_d.ap()[0:NPO, :], zz[:])
            for g in (range(GO) if "3" in PH else []):
                op = pkv.tile([NPO, 2, 2 * C], F32, tag="kvp", name="op")[:, 0, 0:C]
                for cc in range(2):
                    nc.tensor.matmul(
                        op[:], cT_all[:, cc, g * NPO : (g + 1) * NPO],
                        w_sb["wo_s"][:, cc, :],
                        start=(cc == 0), stop=False,
                    )
                bias_mm(op[:], "bo_e", C, NPO)
                o_sb = sb.tile([NPO, C], BF16, tag="o_sb")
                nc.scalar.copy(o_sb[:], op[:])
                nc.gpsimd.dma_start(out_d.ap()[g * NPO : (g + 1) * NPO, :], o_sb[:])

    nc.compile()
    return nc


def _prep_weights(inputs):
    """Host-side weight transforms (all small)."""
    scale = 1.0 / math.sqrt(DK)
    f = lambda k: np.asarray(inputs[k], np.float32)
    tWq, tbq = f("t_Wq") * scale, f("t_bq") * scale
    tWk = f("t_Wk")
    tWv, tbv = f("t_Wv"), f("t_bv")
    tWo, tbo = f("t_Wo"), f("t_bo")
    sWq, sbq = f("s_Wq"), f("s_bq")
    sWk = f("s_Wk")
    sWv, sbv = f("s_Wv"), f("s_bv")
    sWo, sbo = f("s_Wo"), f("s_bo")

    hb = tbv @ tWo + tbo                    # constant part of h_t
    Wq_eff = (tWo @ sWq) * scale
    bq_eff = (hb @ sWq + sbq) * scale
    Wk_eff = tWo @ sWk                      # k bias dropped (softmax-invariant)
    Wv_eff = tWo @ sWv
    cv = hb @ sWv + sbv                     # constant part of v1
    bo_eff = sbo + cv @ sWo

    cm = {
        "wkv_t": _bf16(np.concatenate([tWk, tWv], axis=1).reshape(2, 128, 2 * C)),
        "wq_t": _bf16(tWq.reshape(2, 128, C)),
        "wq_e": _bf16(_pad_cols(Wq_eff).reshape(2, 128, KROW)),
        "wkv_e": _bf16(
            np.concatenate([_pad_cols(Wk_eff), Wv_eff], axis=1).reshape(2, 128, ROW)
        ),
        "wo_s": _bf16(
            sWo.reshape(NH, DK, C)[PERM].reshape(2, 128, C)
        ),
        "bq_t": _bf16(tbq.reshape(1, C)),
        "bq_e": _bf16(_pad_cols(bq_eff).reshape(1, KROW)),
        "bo_e": _bf16(bo_eff.reshape(1, C)),
        "ident": _bf16(np.eye(128, dtype=np.float32)),
        "ones1": _bf16(np.ones((1, 128), np.float32)),
    }
    return cm


def _prep_geometry():
    """Per-core gather indices (local coords) and window masks."""
    masks = np.zeros((CORES, NW, NS * NQ), np.float32)
    for R in range(CR):
        for C4 in range(CC_):
            core = R * CC_ + C4
            for s in range(NS):
                r0, c0 = (s // NS_C) * SQR, (s % NS_C) * SQC
                gr0, gc0 = BR * R + r0, BC * C4 + c0
                wr = gr0 - 2 + np.arange(WR)          # global window rows
                wc = gc0 - 2 + np.arange(WC)
                valid = (wr[:, None] >= 0) & (wr[:, None] < GRID) & \
                        (wc[None, :] >= 0) & (wc[None, :] < GRID)
                qr = gr0 + np.arange(SQR)
                qc = gc0 + np.arange(SQC)
                qrc = np.clip(qr, 2, GRID - 3)
                qcc = np.clip(qc, 2, GRID - 3)
                mrow = (np.abs(wr[:, None] - qrc[None, :]) <= 2)
                mcol = (np.abs(wc[:, None] - qcc[None, :]) <= 2)
                m = (mrow[:, None, :, None] & mcol[None, :, None, :] &
                     valid[:, :, None, None])
                masks[core, :, s * NQ : (s + 1) * NQ] = m.reshape(NW, NQ)
    return _bf16(masks)


def _prep_x(x):
    """x [10000, 5, 256] f32 -> per-core halo-extended channel-major bf16
    chunks [8, G, 128, 2*T*128]."""
    xb = np.asarray(x, np.float32).astype(ml_dtypes.float8_e4m3).reshape(
        GRID, GRID, T, C
    )
    xp = np.zeros((GRID + 4, GRID + 4, T, C), dtype=xb.dtype)
    xp[2 : 2 + GRID, 2 : 2 + GRID] = xb
    out = np.zeros((CORES, G, 128, 2 * T * NP), dtype=xb.dtype)
    for R in range(CR):
        for C4 in range(CC_):
            core = R * CC_ + C4
            blk = xp[BR * R : BR * R + HR, BC * C4 : BC * C4 + HC]  # [54,29,T,C]
            flat = blk.reshape(NH_PIX, T, C)
            flat = np.concatenate(
                [flat, np.zeros((NPAD - NH_PIX, T, C), dtype=xb.dtype)], axis=0
            )
            v = flat.reshape(G, NP, T, 2, 128)
            v = v.transpose(0, 4, 3, 2, 1)      # (g, ch, cc, t, px)
            out[core] = v.reshape(G, 128, 2 * T * NP)
    return out


def _unprep_out(res_list):
    """[8][1250, 256] bf16 -> [10000, 1, 256] f32 global row-major."""
    o = np.stack([np.asarray(r) for r in res_list], axis=0).astype(np.float32)
    v = o.reshape(CR, CC_, BR, BC, C)
    v = v.transpose(0, 2, 1, 3, 4)
    return np.ascontiguousarray(v.reshape(N_FULL, 1, C))


def _make_in_maps(inputs):
    cm = _prep_weights(inputs)
    if "geom" not in _CACHE:
        _CACHE["geom"] = _prep_geometry()
    masks = _CACHE["geom"]
    X = _prep_x(inputs["x"])
    in_maps = []
    for c in range(CORES):
        m = dict(cm)
        m["x"] = X[c]
        m["masks"] = masks[c]
        in_maps.append(m)
    return in_maps


def _get_runner(nc):
    """Build (once) and cache a jitted shard_map callable for the NEFF.

    run_bass_kernel_spmd re-traces and re-jits on every call; caching the
    callable drops warm-call dispatch to the PJRT execute + transfers.
    """
    if "runner" in _CACHE:
        return _CACHE["runner"]
    import jax
    import numpy as jnp_np  # noqa
    from jax.sharding import Mesh, PartitionSpec
    from jax.experimental.shard_map import shard_map
    import concourse.mybir as mb
    from concourse import bass2jax

    bass2jax.install_neuronx_cc_hook()

    in_names, out_names, out_avals, zero_shapes = [], [], [], []
    partition_name = (
        nc.partition_id_tensor.name if nc.partition_id_tensor else None
    )
    for alloc in nc.m.functions[0].allocations:
        if not isinstance(alloc, mb.MemoryLocationSet):
            continue
        name = alloc.memorylocations[0].name
        if alloc.kind == "ExternalInput":
            if name != partition_name:
                in_names.append(name)
        elif alloc.kind == "ExternalOutput":
            shape = tuple(alloc.tensor_shape)
            dtype = mb.dt.np(alloc.dtype)
            out_names.append(name)
            out_avals.append(jax.core.ShapedArray(shape, dtype))
            zero_shapes.append((shape, dtype))
    n_params = len(in_names)
    all_names = list(in_names) + list(out_names)
    if partition_name is not None:
        all_names.append(partition_name)
    donate = tuple(range(n_params, n_params + len(out_names)))

    def _body(*args):
        operands = list(args)
        if partition_name is not None:
            operands.append(bass2jax.partition_id_tensor())
        outs = bass2jax._bass_exec_p.bind(
            *operands,
            out_avals=tuple(out_avals),
            in_names=tuple(all_names),
            out_names=tuple(out_names),
            lowering_input_output_aliases=(),
            sim_require_finite=True,
            sim_require_nnan=True,
            nc=nc,
        )
        return tuple(outs)

    devices = jax.devices()[:CORES]
    mesh = Mesh(np.asarray(devices), ("core",))
    in_specs = (PartitionSpec("core"),) * (n_params + len(out_names))
    out_specs = (PartitionSpec("core"),) * len(out_names)
    sharded = jax.jit(
        shard_map(_body, mesh=mesh, in_specs=in_specs, out_specs=out_specs,
                  check_rep=False),
        donate_argnums=donate, keep_unused=True,
    )

    zfns = [
        jax.jit(
            lambda s=s, dt=dt: jax.numpy.zeros((CORES * s[0], *s[1:]), dt),
            out_shardings=jax.sharding.NamedSharding(mesh, PartitionSpec("core")),
        )
        for s, dt in zero_shapes
    ]
    in_shard = jax.sharding.NamedSharding(mesh, PartitionSpec("core"))

    def run(concat_in):
        args = []
        for n in in_names:
            v = concat_in[n]
            if isinstance(v, tuple):      # (digest, np array): device-cacheable
                key = ("dev", n, v[0])
                if key not in _CACHE:
                    _CACHE[key] = jax.device_put(v[1], in_shard)
                args.append(_CACHE[key])
            else:
                args.append(v)
        zeros = [zf() for zf in zfns]
        outs = sharded(*args, *zeros)
        return {n: outs[i] for i, n in enumerate(out_names)}

    _CACHE["runner"] = run
    return run


def _weights_digest(inputs):
    import hashlib
    h = hashlib.blake2b(digest_size=16)
    for k in sorted(inputs):
        if k not in ("x",):
            h.update(np.ascontiguousarray(inputs[k]).tobytes())
    return h.hexdigest()


def _make_concat_inputs(inputs):
    """Concatenated-along-core-axis input arrays for the cached runner.
    Weight/mask entries are (digest, array) tuples so the runner can keep
    them device-resident across calls."""
    dig = _weights_digest(inputs)
    cm = _prep_weights(inputs)
    if "geom" not in _CACHE:
        _CACHE["geom"] = _prep_geometry()
    masks = _CACHE["geom"]
    X = _prep_x(inputs["x"])
    cat = {}
    for k, v in cm.items():
        full = np.broadcast_to(v, (CORES,) + v.shape).reshape(
            (CORES * v.shape[0],) + v.shape[1:]
        )
        cat[k] = (dig, full)
    cat["x"] = X.reshape(CORES * G, 128, 2 * T * NP)
    cat["masks"] = ("geom", masks.reshape(CORES * NW, NS * NQ))
    return cat


def kernel(**inputs):
    if "nc" not in _CACHE:
        _CACHE["nc"] = _build_graph()
    nc = _CACHE["nc"]
    run = _get_runner(nc)
    cat = _make_concat_inputs(inputs)
    import time as _time
    t0 = _time.perf_counter()
    outs = run(cat)
    out_np = np.asarray(outs["out"])
    _CACHE["last_device_ns"] = (_time.perf_counter() - t0) * 1e9
    o = out_np.reshape(CORES, NLOC, C).astype(np.float32)
    v = o.reshape(CR, CC_, BR, BC, C).transpose(0, 2, 1, 3, 4)
    return np.ascontiguousarray(v.reshape(N_FULL, 1, C))
